# revision 25
# baseline (speedup 1.0000x reference)
"""Trainium2 Bass kernel for nn_GTLayer_84722524880938.

The reference uses .reshape (not transpose) for the attention head split,
which makes attention block-diagonal over 256-row blocks of the sequence:
output rows [256n, 256n+256) depend only on input rows [256n, 256n+256)
(plus the full-length relative-position bias, which is rank-4). The layer
therefore shards perfectly across 8 cores: core c takes 512 contiguous
rows (2 blocks) of batch c//4 and needs no collectives.

Per 256-row block (X = h[b, 256n:256n+256, :]):
  q = X@Wq; k = X@Wk; v = X@Wv            [256, 1024]
  Q = q.reshape(2048, 128); KT = k.reshape(128, 2048); V = v.reshape(2048, 128)
  S = Q@KT/sqrt(128) + (rh[b]@Wrq) @ (rh[b]@Wrk).reshape(4, 2048) / 2
  P = softmax(S, -1);  C = P@V            [2048, 128]
  h_sa = C.reshape(256, 1024) @ Wo
  h1 = LN(h_sa + X);  hf = relu(h1@W1 + b1)@W2 + b2;  out = LN(h1 + hf)

All matmuls run as float32r (full fp32 data, full-rate PE mode). Scores
are exponentiated without max-subtraction (|S| < ~14, far from fp32 exp
overflow). The softmax denominator comes from an extra ones-weight matmul
accumulated alongside P@V.
"""

import sys

sys.path.insert(0, "/opt/trn_rl_repo")

import math

import numpy as np

import concourse.bass as bass
import concourse.mybir as mybir
import concourse.tile as tile
from concourse.bass_utils import run_bass_kernel_spmd
from concourse.masks import make_identity

F32 = mybir.dt.float32
F32R = mybir.dt.float32r
BF16 = mybir.dt.bfloat16

D, FFN, NH, HD, RL = 1024, 4096, 8, 128, 4
B, L = 2, 2048
ROWS = 512  # rows per core
NBLK = 2  # 256-row attention blocks per core
EPS = 1e-5
EXP_SCALE = 1.0 / math.sqrt(HD)  # applied by ACT on scores
RK_SCALE = math.sqrt(HD) / 2.0  # folded into r_k so bias lands as bias/2

MAX_WAITS = 1  # this walrus build allows one semaphore wait per instruction

_cache = {}


def _fix_waits(nc):
    """Split >MAX_WAITS sync waits onto injected same-engine NoOps.

    Engines execute their stream in order, so hoisting excess waits onto
    NoOps placed immediately before the instruction preserves semantics.
    """
    ctr = 0
    for f in nc.m.functions:
        for blk in f.blocks:
            out = []
            changed = False
            for ins in blk.instructions:
                si = ins.sync_info
                waits = list(si.on_wait) if si is not None else []
                if len(waits) > MAX_WAITS:
                    changed = True
                    while len(waits) > MAX_WAITS:
                        chunk, waits = waits[:MAX_WAITS], waits[MAX_WAITS:]
                        ctr += 1
                        nop = mybir.InstNoOp(
                            name=f"waitfix-nop-{ctr}",
                            ins=[],
                            outs=[],
                            sync_info=mybir.SyncInfo(on_wait=chunk, on_update=[]),
                        )
                        nop.engine = ins.engine
                        out.append(nop)
                    ins.sync_info = mybir.SyncInfo(
                        on_wait=waits, on_update=list(si.on_update)
                    )
                out.append(ins)
            if changed:
                blk.instructions = out
    return nc


def _r(ap):
    return ap.bitcast(F32R)


def _fview(base, free_dims, extra_off=0):
    """Rebuild an AP keeping the partition dim, with custom free dims/offset."""
    return bass.AP(
        tensor=base.tensor,
        offset=base.offset + extra_off,
        ap=[list(base.ap[0])] + [list(d) for d in free_dims],
    )


def build_nc(debug=False, repeat=1, phases=None):
    nc = bass.Bass(target_bir_lowering=False)

    x_d = nc.dram_tensor("x", [ROWS, D], F32, kind="ExternalInput")
    rh_d = nc.dram_tensor("rh", [L, RL], F32, kind="ExternalInput")
    wq_d = nc.dram_tensor("Wq", [D, D], F32, kind="ExternalInput")
    wk_d = nc.dram_tensor("Wk", [D, D], F32, kind="ExternalInput")
    wv_d = nc.dram_tensor("Wv", [D, D], F32, kind="ExternalInput")
    wo_d = nc.dram_tensor("Wo", [D, D], F32, kind="ExternalInput")
    wrk_d = nc.dram_tensor("Wrk", [RL, RL], F32, kind="ExternalInput")
    wrq_d = nc.dram_tensor("Wrq", [RL, RL], F32, kind="ExternalInput")
    w1_d = nc.dram_tensor("W1", [D, FFN], F32, kind="ExternalInput")
    b1_d = nc.dram_tensor("b1", [FFN], F32, kind="ExternalInput")
    w2_d = nc.dram_tensor("W2", [FFN, D], F32, kind="ExternalInput")
    b2_d = nc.dram_tensor("b2", [D], F32, kind="ExternalInput")
    g1_d = nc.dram_tensor("g1", [D], F32, kind="ExternalInput")
    be1_d = nc.dram_tensor("be1", [D], F32, kind="ExternalInput")
    g2_d = nc.dram_tensor("g2", [D], F32, kind="ExternalInput")
    be2_d = nc.dram_tensor("be2", [D], F32, kind="ExternalInput")
    out_d = nc.dram_tensor("out", [ROWS, D], F32, kind="ExternalOutput")

    dbg = {}
    if debug:
        dbg["qT"] = nc.dram_tensor("dbg_qT", [128, NH * ROWS], F32, kind="ExternalOutput")
        dbg["KT"] = nc.dram_tensor("dbg_KT", [128, 16, 128], F32, kind="ExternalOutput")
        dbg["V"] = nc.dram_tensor("dbg_V", [128, 16, 128], F32, kind="ExternalOutput")
        dbg["rkR"] = nc.dram_tensor("dbg_rkR", [RL, L], F32, kind="ExternalOutput")
        dbg["rqT"] = nc.dram_tensor("dbg_rqT", [RL, L], F32, kind="ExternalOutput")
        dbg["E"] = nc.dram_tensor("dbg_E", [128, 1024], F32, kind="ExternalOutput")
        dbg["CT"] = nc.dram_tensor("dbg_CT", [128, L], F32, kind="ExternalOutput")
        dbg["h1"] = nc.dram_tensor("dbg_h1", [128, 4, D], F32, kind="ExternalOutput")
        dbg["relu"] = nc.dram_tensor("dbg_relu", [128, ROWS], F32, kind="ExternalOutput")

    ph = phases
    with tile.TileContext(nc, pool_alloc_mode="stack") as tc:
        for _rep in range(repeat):
            _body(nc, tc, locals())

    _fix_waits(nc)
    return nc


def _body(nc, tc, t):
    phases = t["ph"] or {"qkv", "ktv", "attn", "wo", "ffn1", "ffn2"}
    debug = t["debug"]
    dbg = t["dbg"]
    x_d, rh_d, out_d = t["x_d"], t["rh_d"], t["out_d"]

    import contextlib

    ctx = contextlib.ExitStack()
    with ctx:
        # ---- pools ordered by lifetime (longest-lived first) ------------
        singles = ctx.enter_context(tc.tile_pool(name="singles", bufs=1))
        h1T_es = ctx.enter_context(contextlib.ExitStack())
        ct_es = h1T_es.enter_context(contextlib.ExitStack())
        qkv_es = ct_es.enter_context(contextlib.ExitStack())
        kv_es = qkv_es.enter_context(contextlib.ExitStack())

        ident = singles.tile([128, 128], F32)
        make_identity(nc, ident)
        ident_r = singles.tile([128, 128], F32R, name="ident_r")
        nc.vector.tensor_copy(out=ident_r, in_=ident)
        ones_f = singles.tile([128, 128], F32, name="ones_f")
        nc.vector.memset(ones_f, 1.0)
        ones_sb = singles.tile([128, 128], F32R, name="ones_sb")
        nc.vector.tensor_copy(out=ones_sb, in_=ones_f)
        eps_t = singles.tile([128, 1], F32)
        nc.vector.memset(eps_t, EPS)

        def bcast_load(pool, dram, name):
            tl = pool.tile([128, D], F32, name=name, tag=name)
            src = bass.AP(tensor=dram, offset=0, ap=[[0, 128], [1, D]])
            nc.sync.dma_start(out=tl, in_=src)
            return tl

        b1t = singles.tile([128, FFN // 128], F32)
        nc.sync.dma_start(
            out=b1t,
            in_=bass.AP(tensor=t["b1_d"], offset=0, ap=[[1, 128], [128, FFN // 128]]),
        )
        h1_s = singles.tile([128, 4, D], F32R, name="h1_s")  # written after LN1


        h1T_pool = h1T_es.enter_context(tc.tile_pool(name="h1T", bufs=1))
        h1T_s = h1T_pool.tile([128, 8, ROWS], F32R, name="h1T_s")

        ct_pool = ct_es.enter_context(tc.tile_pool(name="ct", bufs=1))
        CT_s = [ct_pool.tile([128, L], F32R, name=f"CTb{b}", tag=f"CTb{b}") for b in range(NBLK)]

        qT_pool = qkv_es.enter_context(tc.tile_pool(name="qT", bufs=1))
        qT_s = qT_pool.tile([128, NH * ROWS], F32R, name="qT_s")
        rqT_s = qT_pool.tile([RL, L], F32R, name="rqT_s")
        rkR_s = qT_pool.tile([RL, L], F32R, name="rkR_s")
        ktv_pool = qkv_es.enter_context(tc.tile_pool(name="ktv", bufs=1))
        KT_sb = [ktv_pool.tile([128, 16, 128], F32R, name=f"KTb{b}", tag=f"KTb{b}") for b in range(NBLK)]
        V_sb = [ktv_pool.tile([128, 16, 128], F32R, name=f"Vb{b}", tag=f"Vb{b}") for b in range(NBLK)]

        kv_pool = kv_es.enter_context(tc.tile_pool(name="kv", bufs=1))
        vT_s = kv_pool.tile([128, NH * ROWS], F32, name="vT_s")
        kstream = kv_es.enter_context(tc.tile_pool(name="kstream", bufs=3))

        # ---- phase 1+2: XT, rel-bias, q/k/v; then KT/V tiles ------------
        with (
            tc.tile_pool(name="xt", bufs=1) as xt_pool,
            tc.tile_pool(name="psT", bufs=2, space="PSUM") as psT,
            tc.tile_pool(name="psMM", bufs=4, space="PSUM") as psMM,
            tc.tile_pool(name="psT2", bufs=2, space="PSUM") as psT2,
            tc.tile_pool(name="wtile", bufs=9) as wpool,
            tc.tile_pool(name="cpy", bufs=3) as cpy,
        ):
            xT_s = xt_pool.tile([128, 8, ROWS], F32R, name="xT_s")
            for rc in range(4):
                xrow = cpy.tile([128, D], F32, tag="xrow", name="xrow")
                nc.sync.dma_start(out=xrow, in_=x_d[rc * 128 : (rc + 1) * 128, :])
                for ct_i in range(8):
                    p = psT.tile([128, 128], F32, tag="pst", name="pT")
                    nc.tensor.transpose(p, xrow[:, ct_i * 128 : (ct_i + 1) * 128], ident)
                    nc.vector.tensor_copy(
                        out=xT_s[:, ct_i, rc * 128 : (rc + 1) * 128], in_=p
                    )

            # rhT [4, 2048] via 16 PE transposes of [128, 4] row tiles
            rh_sb = cpy.tile([128, 16, RL], F32, tag="rh", name="rh_sb")
            nc.sync.dma_start(
                out=rh_sb, in_=rh_d[:, :].rearrange("(a p) u -> p a u", p=128)
            )
            rhT_s = xt_pool.tile([RL, L], F32R, name="rhT_s")
            for a in range(16):
                p = psT.tile([128, 128], F32, tag="pst", name="pT2")
                nc.tensor.transpose(p[:RL, :], rh_sb[:, a, :], ident)
                nc.vector.tensor_copy(
                    out=rhT_s[:, a * 128 : (a + 1) * 128], in_=p[:RL, :]
                )

            # r_qT / r_kT: [4, 2048] = Wr.T @ rh.T
            wr_sb = cpy.tile([RL, 2, RL], F32R, tag="wr", name="wr_sb")
            nc.sync.dma_start(out=wr_sb[:, 0, :], in_=t["wrq_d"][:, :].bitcast(F32R))
            nc.sync.dma_start(out=wr_sb[:, 1, :], in_=t["wrk_d"][:, :].bitcast(F32R))
            rkT_s = xt_pool.tile([RL, L], F32R, name="rkT_s")
            for half in range(4):
                sl = slice(half * 512, (half + 1) * 512)
                pq = psMM.tile([128, 512], F32, tag="qkv", name="pq")[:RL, :]
                nc.tensor.matmul(
                    pq, _r(wr_sb[:, 0, :]), _r(rhT_s[:, sl]), start=True, stop=True
                )
                nc.vector.tensor_copy(out=rqT_s[:, sl], in_=pq)
                pk = psMM.tile([128, 512], F32, tag="qkv", name="pk")[:RL, :]
                nc.tensor.matmul(
                    pk, _r(wr_sb[:, 1, :]), _r(rhT_s[:, sl]), start=True, stop=True
                )
                nc.vector.tensor_scalar_mul(out=rkT_s[:, sl], in0=pk, scalar1=RK_SCALE)

            # rkR[t, 4i+u] = rkT[u, 512t+i]  (reshape(4, 2048) of r_k)
            for tt in range(4):
                for u in range(RL):
                    nc.sync.dma_start(
                        out=_fview(rkR_s[tt : tt + 1, :], [[RL, 512]], u),
                        in_=rkT_s[u : u + 1, tt * 512 : (tt + 1) * 512],
                    )

            if debug:
                nc.sync.dma_start(out=dbg["rkR"][:, :], in_=rkR_s.bitcast(F32))
                nc.sync.dma_start(out=dbg["rqT"][:, :], in_=rqT_s.bitcast(F32))

            # q and v full [128, 8, ROWS]; weights loaded as [128, 512] half-rows
            for w_d, dest in ((t["wq_d"], qT_s), (t["wv_d"], vT_s)) if "qkv" in phases else ():
                for half in range(2):
                    wrows = [None] * 8
                    for ci in range(8):
                        wrow = wpool.tile([128, 512], F32R, tag="w", name="wrow")
                        nc.sync.dma_start(
                            out=wrow,
                            in_=w_d[
                                ci * 128 : (ci + 1) * 128,
                                half * 512 : (half + 1) * 512,
                            ].bitcast(F32R),
                        )
                        wrows[ci] = wrow
                    for col in range(4):
                        co = half * 4 + col
                        pm = psMM.tile([128, ROWS], F32, tag="qkv", name="pm")
                        for ci in range(8):
                            nc.tensor.matmul(
                                pm,
                                _r(wrows[ci][:, col * 128 : (col + 1) * 128]),
                                _r(xT_s[:, ci, :]),
                                start=(ci == 0),
                                stop=(ci == 7),
                            )
                        nc.vector.tensor_copy(
                            out=_fview(dest[:, :], [[8, ROWS]], co), in_=pm
                        )

            # k per-co streaming: each co slice feeds only KT tiles t%8==co
            for half in range(2 if "qkv" in phases else 0):
                wrows = [None] * 8
                for ci in range(8):
                    wrow = wpool.tile([128, 512], F32R, tag="w", name="wkrow")
                    nc.sync.dma_start(
                        out=wrow,
                        in_=t["wk_d"][
                            ci * 128 : (ci + 1) * 128, half * 512 : (half + 1) * 512
                        ].bitcast(F32R),
                    )
                    wrows[ci] = wrow
                for col in range(4):
                    co = half * 4 + col
                    pm = psMM.tile([128, ROWS], F32, tag="qkv", name="pmk")
                    for ci in range(8):
                        nc.tensor.matmul(
                            pm,
                            _r(wrows[ci][:, col * 128 : (col + 1) * 128]),
                            _r(xT_s[:, ci, :]),
                            start=(ci == 0),
                            stop=(ci == 7),
                        )
                    kco = kstream.tile([128, ROWS], F32, tag="kco", name="kco")
                    nc.vector.tensor_copy(out=kco, in_=pm)
                    for b in range(NBLK):
                        for tt in (co, co + 8):
                            # KT_t^T[mm, d] = k[256b + 2d + (t>=8), 128co + mm]
                            kt_view = _fview(
                                kco[:, :], [[2, 128]], 256 * b + (1 if tt >= 8 else 0)
                            )
                            p = psT2.tile([128, 128], F32, tag="pst2", name="pKT")
                            nc.tensor.transpose(p, kt_view, ident)
                            nc.vector.tensor_copy(out=KT_sb[b][:, tt, :], in_=p)

            # V tiles from vT_s
            for b in range(NBLK if "ktv" in phases else 0):
                for tt in range(16):
                    # V_t^T[e, 8a+j] = vT_s[e, j, 256b + 16t + a]
                    v_view = _fview(
                        vT_s[:, :], [[1, 128]], 8 * (256 * b + 16 * tt)
                    )
                    pv = psT2.tile([128, 128], F32, tag="pst2", name="pV")
                    nc.tensor.transpose(pv, v_view, ident)
                    nc.vector.tensor_copy(out=V_sb[b][:, tt, :], in_=pv)
            if debug:
                nc.sync.dma_start(out=dbg["qT"][:, :], in_=qT_s.bitcast(F32))
                nc.sync.dma_start(out=dbg["KT"][:, :, :], in_=KT_sb[0].bitcast(F32))
                nc.sync.dma_start(out=dbg["V"][:, :, :], in_=V_sb[0].bitcast(F32))
        kv_es.close()  # vT/k-stream dead once KT/V tiles exist

        # Wo preload: region reuses the kv pool space (freed at P3 end), so
        # this 4MB DMA overlaps the whole attention phase. Lives in qkv_es,
        # whose close moves to after the Wo phase to keep LIFO order.
        wopool = qkv_es.enter_context(tc.tile_pool(name="wotile", bufs=1))
        wo_s = wopool.tile([128, 8, D], F32R, name="wo_s")
        nc.sync.dma_start(
            out=wo_s,
            in_=t["wo_d"][:, :].rearrange("(j p) n -> p j n", p=128).bitcast(F32R),
        )
        g1b = bcast_load(wopool, t["g1_d"], "g1b")
        be1b = bcast_load(wopool, t["be1_d"], "be1b")

        def layer_norm(dest, pre, gb, bb, pool):
            """dest = LN(pre) * gb + bb ; pre is [128, 1024] SBUF.

            The gamma/beta apply is split by column half across gpsimd and
            DVE so the two chains run in parallel.
            """
            st = pool.tile([128, 2, 6], F32, tag="bnst", name="st")
            nc.vector.bn_stats(out=st[:, 0, :], in_=pre[:, 0:512])
            nc.vector.bn_stats(out=st[:, 1, :], in_=pre[:, 512:1024])
            mv = pool.tile([128, 2], F32, tag="bnmv", name="mv")
            nc.vector.bn_aggr(out=mv, in_=st)
            rstd = pool.tile([128, 1], F32, tag="rstd", name="rstd")
            nc.scalar.activation(
                out=rstd,
                in_=mv[:, 1:2],
                func=mybir.ActivationFunctionType.Sqrt,
                bias=eps_t,
            )
            nc.vector.reciprocal(out=rstd, in_=rstd)
            xn = pool.tile([128, D], F32, tag="xn", name="xn")
            nc.vector.tensor_scalar(
                out=xn,
                in0=pre,
                scalar1=mv[:, 0:1],
                scalar2=rstd,
                op0=mybir.AluOpType.subtract,
                op1=mybir.AluOpType.mult,
            )
            cs1, cs0 = slice(512, 1024), slice(0, 512)
            nc.gpsimd.tensor_mul(out=xn[:, cs1], in0=xn[:, cs1], in1=gb[:, cs1])
            nc.vector.tensor_mul(out=xn[:, cs0], in0=xn[:, cs0], in1=gb[:, cs0])
            nc.vector.tensor_add(out=dest[:, cs0], in0=xn[:, cs0], in1=bb[:, cs0])
            nc.vector.tensor_add(out=dest[:, cs1], in0=xn[:, cs1], in1=bb[:, cs1])

        # ---- phases 4+5 fused: attention with per-lh Wo + LN1 -----------
        # Wide [128,1024] score tiles halve the exp count (one Act exp per
        # key tile instead of two), cutting Act's fixed per-instruction
        # overhead — Act was the attention rate-limiter. PSUM: S 2x2 + C 2
        # + D 2 banks; Wo's accumulators borrow the D banks between lh
        # rounds (pD is dead after the reciprocal).
        with (
            tc.tile_pool(name="psS", bufs=2, space="PSUM") as psS,
            tc.tile_pool(name="psC", bufs=1, space="PSUM") as psC,
            tc.tile_pool(name="psD", bufs=1, space="PSUM") as psD,
            tc.tile_pool(name="epool", bufs=3) as epool,
            tc.tile_pool(name="inv", bufs=2) as invp,
            tc.tile_pool(name="lnp", bufs=2) as lnp,
        ):
            def wo_ln1(b, rc2):
                a = 2 * b + rc2  # core row-chunk index
                xrow = lnp.tile([128, D], F32, tag="xrow2", name="xrow2")
                nc.sync.dma_start(out=xrow, in_=x_d[a * 128 : (a + 1) * 128, :])
                pre = lnp.tile([128, D], F32, tag="pre", name="pre")
                for nchunk in range(2):
                    ph = psD.tile([128, 512], F32, tag="pd", name="ph")
                    for j in range(8):
                        ctx_view = _fview(CT_s[b][:, :], [[8, 128]], 1024 * rc2 + j)
                        nc.tensor.matmul(
                            ph,
                            _r(ctx_view),
                            _r(wo_s[:, j, nchunk * 512 : (nchunk + 1) * 512]),
                            start=(j == 0),
                            stop=(j == 7),
                        )
                    nc.vector.tensor_add(
                        out=pre[:, nchunk * 512 : (nchunk + 1) * 512],
                        in0=ph,
                        in1=xrow[:, nchunk * 512 : (nchunk + 1) * 512],
                    )
                layer_norm(h1_s[:, a, :], pre, g1b, be1b, lnp)

            for b in range(NBLK if "attn" in phases else 0):
                for lh in range(2):
                    pC = psC.tile([128, 1024], F32, tag="pc", name="pC")
                    pD = psD.tile([128, 1024], F32, tag="pd", name="pD")
                    for tt in range(16):
                        e_t = epool.tile([128, 1024], F32R, tag="e", name="e_t")
                        pS = psS.tile([128, 1024], F32, tag="ps", name="pS")
                        for q in range(2):
                            lq = slice(q * 512, (q + 1) * 512)
                            # l = 1024*lh + 512*q + 8r + j ; r0 = 128*lh + 64*q
                            off = 8 * (256 * b + 128 * lh + 64 * q)
                            qt_view = qT_s[:, off : off + 512]
                            nc.tensor.matmul(
                                pS[:, lq],
                                _r(KT_sb[b][:, tt, :]),
                                _r(qt_view),
                                start=True,
                                stop=False,
                            )
                            nc.tensor.matmul(
                                pS[:, lq],
                                _r(rkR_s[:, tt * 128 : (tt + 1) * 128]),
                                _r(rqT_s[:, 1024 * lh + 512 * q :][:, :512]),
                                start=False,
                                stop=True,
                            )
                        nc.scalar.activation(
                            out=e_t,
                            in_=pS,
                            func=mybir.ActivationFunctionType.Exp,
                            scale=EXP_SCALE,
                        )
                        if debug and b == 0 and lh == 0 and tt == 0:
                            nc.sync.dma_start(out=dbg["E"][:, :], in_=e_t.bitcast(F32))
                        for q in range(2):
                            lq = slice(q * 512, (q + 1) * 512)
                            nc.tensor.matmul(
                                pC[:, lq],
                                _r(V_sb[b][:, tt, :]),
                                _r(e_t[:, lq]),
                                start=(tt == 0),
                                stop=(tt == 15),
                            )
                            nc.tensor.matmul(
                                pD[:, lq],
                                _r(ones_sb),
                                _r(e_t[:, lq]),
                                start=(tt == 0),
                                stop=(tt == 15),
                            )
                    inv_t = invp.tile([128, 1024], F32, tag="inv", name="inv_t")
                    nc.vector.reciprocal(out=inv_t, in_=pD)
                    nc.vector.tensor_mul(
                        out=CT_s[b][:, 1024 * lh : 1024 * (lh + 1)],
                        in0=pC,
                        in1=inv_t,
                    )
                    if "wo" in phases:
                        wo_ln1(b, lh)
            if debug:
                nc.sync.dma_start(out=dbg["CT"][:, :], in_=CT_s[0].bitcast(F32))
                nc.sync.dma_start(out=dbg["h1"][:, :, :], in_=h1_s.bitcast(F32))
        qkv_es.close()  # qT/rel/KT/V/Wo dead after Wo+LN1
        ct_es.close()  # CT dead after Wo

        # FFN pools open before phase 6 so the first W1 half-group (and the
        # W2/relu regions) are live early; W1 hg=0 loads under the h1T phase.
        ffn2_es = ctx.enter_context(contextlib.ExitStack())
        w2pool = ffn2_es.enter_context(tc.tile_pool(name="w2tile", bufs=3))
        rkeep = ffn2_es.enter_context(tc.tile_pool(name="rkeep", bufs=32))
        w1_es = ctx.enter_context(contextlib.ExitStack())
        w1pool = w1_es.enter_context(tc.tile_pool(name="w1tile", bufs=14))
        rkeep_tiles = {}
        w1half = {}

        def w1_load(hg):
            tiles = [None] * 8
            for ci in range(8):
                wt = w1pool.tile([128, 512], F32R, tag="w1", name="w1t")
                nc.sync.dma_start(
                    out=wt,
                    in_=t["w1_d"][
                        ci * 128 : (ci + 1) * 128, hg * 512 : (hg + 1) * 512
                    ].bitcast(F32R),
                )
                tiles[ci] = wt
            w1half[hg] = tiles

        if "ffn1" in phases:
            w1_load(0)

        # ---- phase 6: h1T -----------------------------------------------
        with tc.tile_pool(name="psT3", bufs=2, space="PSUM") as psT3:
            for ct_i in range(8):
                for a in range(4):
                    p = psT3.tile([128, 128], F32R, tag="pst3", name="pH")
                    nc.tensor.transpose(
                        p, h1_s[:, a, ct_i * 128 : (ct_i + 1) * 128], ident_r
                    )
                    nc.vector.tensor_copy(
                        out=h1T_s[:, ct_i, a * 128 : (a + 1) * 128], in_=p
                    )

        # h1_s += b2 (broadcast) so FFN2's identity-inject seeds h1+b2.
        # Runs after the transposes read pristine h1; hidden under FFN1.
        b2b = bcast_load(singles, t["b2_d"], "b2b")
        for a in range(4):
            nc.vector.tensor_add(
                out=h1_s[:, a, :], in0=h1_s[:, a, :], in1=b2b
            )


        # ---- phase 7: FFN1 + relu (kept in SBUF as bf16) ----------------
        # W1 streams in 4-f half-groups ([128, 512] per ci) so buffers free
        # incrementally and the DMA stays ahead. All 32 relu tiles live in
        # SBUF as bf16 (4MB) — no DRAM bounce.
        with tc.tile_pool(name="psF1", bufs=4, space="PSUM") as psF1:
            w2g_pre = {}
            for f in range(32 if "ffn1" in phases else 0):
                hg, fl = f // 4, f % 4
                if fl == 0 and hg + 1 < 8:
                    w1_load(hg + 1)
                if f == 28:
                    # prefetch FFN2's first two weight groups while the last
                    # W1 half-groups stream
                    for g in range(2):
                        wg = w2pool.tile([128, 4, D], F32R, tag="w2", name="w2g")
                        nc.sync.dma_start(
                            out=wg,
                            in_=t["w2_d"][g * 512 : (g + 1) * 512, :]
                            .rearrange("(g p) c -> p g c", p=128)
                            .bitcast(F32R),
                        )
                        w2g_pre[g] = wg
                pm = psF1.tile([128, ROWS], F32, tag="psf1", name="pF")
                for ci in range(8):
                    nc.tensor.matmul(
                        pm,
                        _r(w1half[hg][ci][:, fl * 128 : (fl + 1) * 128]),
                        _r(h1T_s[:, ci, :]),
                        start=(ci == 0),
                        stop=(ci == 7),
                    )
                rt = rkeep.tile([128, ROWS], F32R, tag="rkeep", name="rk")
                rkeep_tiles[f] = rt
                nc.scalar.activation(
                    out=rt,
                    in_=pm,
                    func=mybir.ActivationFunctionType.Relu,
                    bias=b1t[:, f : f + 1],
                )
        w1_es.close()  # free W1 buffers before FFN2's pools allocate
        # ---- phase 8: FFN2 + residual + LN2 + store ---------------------
        # The h1+b2 residual is injected into each PSUM accumulator by an
        # identity-matmul at accumulation start (h1_s holds h1+b2 by then),
        # so the epilogue reads LN2 stats straight off PSUM. The last weight
        # group runs a-outer so accumulators finish staggered and the four
        # LN2 chains pipeline behind the remaining matmuls.
        with (
            tc.tile_pool(name="psF2", bufs=1, space="PSUM") as psF2,
            tc.tile_pool(name="ln2p", bufs=2) as ln2p,
            tc.tile_pool(name="outp", bufs=2) as outp,
        ):
            g2b = bcast_load(ln2p, t["g2_d"], "g2b")
            be2b = bcast_load(ln2p, t["be2_d"], "be2b")
            pacc = [
                psF2.tile([128, 512], F32, tag=f"psf2_{i}", name=f"psf2_{i}")
                for i in range(8)
            ]
            if "ffn2" in phases:
                for i in range(8):
                    a, cchunk = i // 2, i % 2
                    cs = slice(cchunk * 512, (cchunk + 1) * 512)
                    nc.tensor.matmul(
                        pacc[i], ident_r, h1_s[:, a, cs], start=True, stop=False
                    )

            def f2_mm(f, a, w2g):
                fl = f % 4
                for cchunk in range(2):
                    nc.tensor.matmul(
                        pacc[a * 2 + cchunk],
                        rkeep_tiles[f][:, a * 128 : (a + 1) * 128],
                        _r(w2g[:, fl, cchunk * 512 : (cchunk + 1) * 512]),
                        start=False,
                        stop=(f == 31),
                    )

            def ln2_epilogue(a):
                st = ln2p.tile([128, 2, 6], F32, tag="bnst", name="st2")
                nc.vector.bn_stats(out=st[:, 0, :], in_=pacc[a * 2])
                nc.vector.bn_stats(out=st[:, 1, :], in_=pacc[a * 2 + 1])
                mv = ln2p.tile([128, 2], F32, tag="bnmv", name="mv2")
                nc.vector.bn_aggr(out=mv, in_=st)
                rstd = ln2p.tile([128, 1], F32, tag="rstd", name="rstd2")
                nc.scalar.activation(
                    out=rstd,
                    in_=mv[:, 1:2],
                    func=mybir.ActivationFunctionType.Sqrt,
                    bias=eps_t,
                )
                nc.vector.reciprocal(out=rstd, in_=rstd)
                o_t = outp.tile([128, D], F32, tag="o", name="o_t")
                # normalize on DVE (PSUM access); gamma/beta apply split
                # across gpsimd (chunk 1) and DVE (chunk 0) to run in parallel
                for cchunk in range(2):
                    cs = slice(cchunk * 512, (cchunk + 1) * 512)
                    nc.vector.tensor_scalar(
                        out=o_t[:, cs],
                        in0=pacc[a * 2 + cchunk],
                        scalar1=mv[:, 0:1],
                        scalar2=rstd,
                        op0=mybir.AluOpType.subtract,
                        op1=mybir.AluOpType.mult,
                    )
                for cchunk, eng in ((1, nc.gpsimd), (0, nc.vector)):
                    cs = slice(cchunk * 512, (cchunk + 1) * 512)
                    eng.tensor_mul(out=o_t[:, cs], in0=o_t[:, cs], in1=g2b[:, cs])
                    eng.tensor_add(out=o_t[:, cs], in0=o_t[:, cs], in1=be2b[:, cs])
                nc.sync.dma_start(out=out_d[a * 128 : (a + 1) * 128, :], in_=o_t)

            for f in range(24 if "ffn2" in phases else 0):
                fg, fl = f // 4, f % 4
                if fl == 0:
                    if fg in w2g_pre:
                        w2g = w2g_pre[fg]
                    else:
                        w2g = w2pool.tile([128, 4, D], F32R, tag="w2", name="w2g")
                        nc.sync.dma_start(
                            out=w2g,
                            in_=t["w2_d"][fg * 512 : (fg + 1) * 512, :]
                            .rearrange("(g p) c -> p g c", p=128)
                            .bitcast(F32R),
                        )
                for a in range(4):
                    f2_mm(f, a, w2g)
            if "ffn2" in phases:
                # last two weight groups run a-outer: each row-chunk's
                # accumulators stop early and its LN2 chain overlaps the rest
                w2g_last = {}
                for fg in (6, 7):
                    wg = w2pool.tile([128, 4, D], F32R, tag="w2", name="w2g")
                    nc.sync.dma_start(
                        out=wg,
                        in_=t["w2_d"][fg * 512 : (fg + 1) * 512, :]
                        .rearrange("(g p) c -> p g c", p=128)
                        .bitcast(F32R),
                    )
                    w2g_last[fg] = wg
                for a in range(4):
                    for f in range(24, 32):
                        f2_mm(f, a, w2g_last[f // 4])
                    ln2_epilogue(a)


def _get_nc(debug=False):
    key = ("dbg" if debug else "main")
    if key not in _cache:
        _cache[key] = build_nc(debug)
    return _cache[key]


def kernel(**inputs):
    h = np.ascontiguousarray(np.asarray(inputs["h"], dtype=np.float32))
    rh = np.ascontiguousarray(np.asarray(inputs["rh"], dtype=np.float32))
    weights = {
        k: np.ascontiguousarray(np.asarray(inputs[k], dtype=np.float32))
        for k in (
            "Wq", "Wk", "Wv", "Wo", "Wrk", "Wrq",
            "W1", "b1", "W2", "b2", "g1", "be1", "g2", "be2",
        )
    }
    in_maps = []
    for c in range(8):
        b, r0 = c // 4, 512 * (c % 4)
        m = {"x": h[b, r0 : r0 + 512, :], "rh": rh[b]}
        m.update(weights)
        in_maps.append(m)

    nc = _get_nc()
    res = run_bass_kernel_spmd(nc, in_maps, core_ids=list(range(8)))
    out = np.empty((B, L, D), dtype=np.float32)
    for c in range(8):
        b, r0 = c // 4, 512 * (c % 4)
        out[b, r0 : r0 + 512, :] = res.results[c]["out"]
    return out



# revision 26
# speedup vs baseline: 1.0284x; 1.0284x over previous
"""Trainium2 Bass kernel for nn_GTLayer_84722524880938.

The reference uses .reshape (not transpose) for the attention head split,
which makes attention block-diagonal over 256-row blocks of the sequence:
output rows [256n, 256n+256) depend only on input rows [256n, 256n+256)
(plus the full-length relative-position bias, which is rank-4). The layer
therefore shards perfectly across 8 cores: core c takes 512 contiguous
rows (2 blocks) of batch c//4 and needs no collectives.

Per 256-row block (X = h[b, 256n:256n+256, :]):
  q = X@Wq; k = X@Wk; v = X@Wv            [256, 1024]
  Q = q.reshape(2048, 128); KT = k.reshape(128, 2048); V = v.reshape(2048, 128)
  S = Q@KT/sqrt(128) + (rh[b]@Wrq) @ (rh[b]@Wrk).reshape(4, 2048) / 2
  P = softmax(S, -1);  C = P@V            [2048, 128]
  h_sa = C.reshape(256, 1024) @ Wo
  h1 = LN(h_sa + X);  hf = relu(h1@W1 + b1)@W2 + b2;  out = LN(h1 + hf)

All matmuls run as float32r (full fp32 data, full-rate PE mode). Scores
are exponentiated without max-subtraction (|S| < ~14, far from fp32 exp
overflow). The softmax denominator comes from an extra ones-weight matmul
accumulated alongside P@V.
"""

import sys

sys.path.insert(0, "/opt/trn_rl_repo")

import math

import numpy as np

import concourse.bass as bass
import concourse.mybir as mybir
import concourse.tile as tile
from concourse.bass_utils import run_bass_kernel_spmd
from concourse.masks import make_identity

F32 = mybir.dt.float32
F32R = mybir.dt.float32r
BF16 = mybir.dt.bfloat16

D, FFN, NH, HD, RL = 1024, 4096, 8, 128, 4
B, L = 2, 2048
ROWS = 512  # rows per core
NBLK = 2  # 256-row attention blocks per core
EPS = 1e-5
EXP_SCALE = 1.0 / math.sqrt(HD)  # applied by ACT on scores
RK_SCALE = math.sqrt(HD) / 2.0  # folded into r_k so bias lands as bias/2

MAX_WAITS = 1  # this walrus build allows one semaphore wait per instruction

_cache = {}


def _fix_waits(nc):
    """Split >MAX_WAITS sync waits onto injected same-engine NoOps.

    Engines execute their stream in order, so hoisting excess waits onto
    NoOps placed immediately before the instruction preserves semantics.
    """
    ctr = 0
    for f in nc.m.functions:
        for blk in f.blocks:
            out = []
            changed = False
            for ins in blk.instructions:
                si = ins.sync_info
                waits = list(si.on_wait) if si is not None else []
                if len(waits) > MAX_WAITS:
                    changed = True
                    while len(waits) > MAX_WAITS:
                        chunk, waits = waits[:MAX_WAITS], waits[MAX_WAITS:]
                        ctr += 1
                        nop = mybir.InstNoOp(
                            name=f"waitfix-nop-{ctr}",
                            ins=[],
                            outs=[],
                            sync_info=mybir.SyncInfo(on_wait=chunk, on_update=[]),
                        )
                        nop.engine = ins.engine
                        out.append(nop)
                    ins.sync_info = mybir.SyncInfo(
                        on_wait=waits, on_update=list(si.on_update)
                    )
                out.append(ins)
            if changed:
                blk.instructions = out
    return nc


def _r(ap):
    return ap.bitcast(F32R)


def _fview(base, free_dims, extra_off=0):
    """Rebuild an AP keeping the partition dim, with custom free dims/offset."""
    return bass.AP(
        tensor=base.tensor,
        offset=base.offset + extra_off,
        ap=[list(base.ap[0])] + [list(d) for d in free_dims],
    )


def build_nc(debug=False, repeat=1, phases=None):
    nc = bass.Bass(target_bir_lowering=False)

    x_d = nc.dram_tensor("x", [ROWS, D], F32, kind="ExternalInput")
    rh_d = nc.dram_tensor("rh", [L, RL], F32, kind="ExternalInput")
    wq_d = nc.dram_tensor("Wq", [D, D], F32, kind="ExternalInput")
    wk_d = nc.dram_tensor("Wk", [D, D], F32, kind="ExternalInput")
    wv_d = nc.dram_tensor("Wv", [D, D], F32, kind="ExternalInput")
    wo_d = nc.dram_tensor("Wo", [D, D], F32, kind="ExternalInput")
    wrk_d = nc.dram_tensor("Wrk", [RL, RL], F32, kind="ExternalInput")
    wrq_d = nc.dram_tensor("Wrq", [RL, RL], F32, kind="ExternalInput")
    w1_d = nc.dram_tensor("W1", [D, FFN], F32, kind="ExternalInput")
    b1_d = nc.dram_tensor("b1", [FFN], F32, kind="ExternalInput")
    w2_d = nc.dram_tensor("W2", [FFN, D], F32, kind="ExternalInput")
    b2_d = nc.dram_tensor("b2", [D], F32, kind="ExternalInput")
    g1_d = nc.dram_tensor("g1", [D], F32, kind="ExternalInput")
    be1_d = nc.dram_tensor("be1", [D], F32, kind="ExternalInput")
    g2_d = nc.dram_tensor("g2", [D], F32, kind="ExternalInput")
    be2_d = nc.dram_tensor("be2", [D], F32, kind="ExternalInput")
    out_d = nc.dram_tensor("out", [ROWS, D], F32, kind="ExternalOutput")

    dbg = {}
    if debug:
        dbg["qT"] = nc.dram_tensor("dbg_qT", [128, NH * ROWS], F32, kind="ExternalOutput")
        dbg["KT"] = nc.dram_tensor("dbg_KT", [128, 16, 128], F32, kind="ExternalOutput")
        dbg["V"] = nc.dram_tensor("dbg_V", [128, 16, 128], F32, kind="ExternalOutput")
        dbg["rkR"] = nc.dram_tensor("dbg_rkR", [RL, L], F32, kind="ExternalOutput")
        dbg["rqT"] = nc.dram_tensor("dbg_rqT", [RL, L], F32, kind="ExternalOutput")
        dbg["E"] = nc.dram_tensor("dbg_E", [128, 1024], F32, kind="ExternalOutput")
        dbg["CT"] = nc.dram_tensor("dbg_CT", [128, L], F32, kind="ExternalOutput")
        dbg["h1"] = nc.dram_tensor("dbg_h1", [128, 4, D], F32, kind="ExternalOutput")
        dbg["relu"] = nc.dram_tensor("dbg_relu", [128, ROWS], F32, kind="ExternalOutput")

    ph = phases
    with tile.TileContext(nc, pool_alloc_mode="stack") as tc:
        for _rep in range(repeat):
            _body(nc, tc, locals())

    _fix_waits(nc)
    return nc


def _body(nc, tc, t):
    phases = t["ph"] or {"qkv", "ktv", "attn", "wo", "ffn1", "ffn2"}
    debug = t["debug"]
    dbg = t["dbg"]
    x_d, rh_d, out_d = t["x_d"], t["rh_d"], t["out_d"]

    import contextlib

    ctx = contextlib.ExitStack()
    with ctx:
        # ---- pools ordered by lifetime (longest-lived first) ------------
        singles = ctx.enter_context(tc.tile_pool(name="singles", bufs=1))
        h1T_es = ctx.enter_context(contextlib.ExitStack())
        ct_es = h1T_es.enter_context(contextlib.ExitStack())
        qkv_es = ct_es.enter_context(contextlib.ExitStack())
        kv_es = qkv_es.enter_context(contextlib.ExitStack())

        ident = singles.tile([128, 128], F32)
        make_identity(nc, ident)
        ident_r = singles.tile([128, 128], F32R, name="ident_r")
        nc.vector.tensor_copy(out=ident_r, in_=ident)
        ones_f = singles.tile([128, 128], F32, name="ones_f")
        nc.vector.memset(ones_f, 1.0)
        ones_sb = singles.tile([128, 128], F32R, name="ones_sb")
        nc.vector.tensor_copy(out=ones_sb, in_=ones_f)
        eps_t = singles.tile([128, 1], F32)
        nc.vector.memset(eps_t, EPS)

        def bcast_load(pool, dram, name):
            tl = pool.tile([128, D], F32, name=name, tag=name)
            src = bass.AP(tensor=dram, offset=0, ap=[[0, 128], [1, D]])
            nc.sync.dma_start(out=tl, in_=src)
            return tl

        b1t = singles.tile([128, FFN // 128], F32)
        nc.sync.dma_start(
            out=b1t,
            in_=bass.AP(tensor=t["b1_d"], offset=0, ap=[[1, 128], [128, FFN // 128]]),
        )
        h1_s = singles.tile([128, 4, D], F32R, name="h1_s")  # written after LN1


        h1T_pool = h1T_es.enter_context(tc.tile_pool(name="h1T", bufs=1))
        h1T_s = h1T_pool.tile([128, 8, ROWS], F32R, name="h1T_s")

        ct_pool = ct_es.enter_context(tc.tile_pool(name="ct", bufs=1))
        CT_s = [ct_pool.tile([128, L], F32R, name=f"CTb{b}", tag=f"CTb{b}") for b in range(NBLK)]

        qT_pool = qkv_es.enter_context(tc.tile_pool(name="qT", bufs=1))
        qT_s = qT_pool.tile([128, NH * ROWS], F32R, name="qT_s")
        rqT_s = qT_pool.tile([RL, L], F32R, name="rqT_s")
        rkR_s = qT_pool.tile([RL, L], F32R, name="rkR_s")
        ktv_pool = qkv_es.enter_context(tc.tile_pool(name="ktv", bufs=1))
        KT_sb = [ktv_pool.tile([128, 16, 128], F32R, name=f"KTb{b}", tag=f"KTb{b}") for b in range(NBLK)]
        V_sb = [ktv_pool.tile([128, 16, 128], F32R, name=f"Vb{b}", tag=f"Vb{b}") for b in range(NBLK)]

        kv_pool = kv_es.enter_context(tc.tile_pool(name="kv", bufs=1))
        vT_s = kv_pool.tile([128, NH * ROWS], F32, name="vT_s")
        kstream = kv_es.enter_context(tc.tile_pool(name="kstream", bufs=3))

        # ---- phase 1+2: XT, rel-bias, q/k/v; then KT/V tiles ------------
        with (
            tc.tile_pool(name="xt", bufs=1) as xt_pool,
            tc.tile_pool(name="psT", bufs=2, space="PSUM") as psT,
            tc.tile_pool(name="psMM", bufs=4, space="PSUM") as psMM,
            tc.tile_pool(name="psT2", bufs=2, space="PSUM") as psT2,
            tc.tile_pool(name="wtile", bufs=9) as wpool,
            tc.tile_pool(name="cpy", bufs=3) as cpy,
        ):
            xT_s = xt_pool.tile([128, 8, ROWS], F32R, name="xT_s")
            for rc in range(4):
                xrow = cpy.tile([128, D], F32, tag="xrow", name="xrow")
                nc.sync.dma_start(out=xrow, in_=x_d[rc * 128 : (rc + 1) * 128, :])
                for ct_i in range(8):
                    p = psT.tile([128, 128], F32, tag="pst", name="pT")
                    nc.tensor.transpose(p, xrow[:, ct_i * 128 : (ct_i + 1) * 128], ident)
                    nc.vector.tensor_copy(
                        out=xT_s[:, ct_i, rc * 128 : (rc + 1) * 128], in_=p
                    )

            # rhT [4, 2048] via 16 PE transposes of [128, 4] row tiles
            rh_sb = cpy.tile([128, 16, RL], F32, tag="rh", name="rh_sb")
            nc.sync.dma_start(
                out=rh_sb, in_=rh_d[:, :].rearrange("(a p) u -> p a u", p=128)
            )
            rhT_s = xt_pool.tile([RL, L], F32R, name="rhT_s")
            for a in range(16):
                p = psT.tile([128, 128], F32, tag="pst", name="pT2")
                nc.tensor.transpose(p[:RL, :], rh_sb[:, a, :], ident)
                nc.vector.tensor_copy(
                    out=rhT_s[:, a * 128 : (a + 1) * 128], in_=p[:RL, :]
                )

            # r_qT / r_kT: [4, 2048] = Wr.T @ rh.T
            wr_sb = cpy.tile([RL, 2, RL], F32R, tag="wr", name="wr_sb")
            nc.sync.dma_start(out=wr_sb[:, 0, :], in_=t["wrq_d"][:, :].bitcast(F32R))
            nc.sync.dma_start(out=wr_sb[:, 1, :], in_=t["wrk_d"][:, :].bitcast(F32R))
            rkT_s = xt_pool.tile([RL, L], F32R, name="rkT_s")
            for half in range(4):
                sl = slice(half * 512, (half + 1) * 512)
                pq = psMM.tile([128, 512], F32, tag="qkv", name="pq")[:RL, :]
                nc.tensor.matmul(
                    pq, _r(wr_sb[:, 0, :]), _r(rhT_s[:, sl]), start=True, stop=True
                )
                nc.vector.tensor_copy(out=rqT_s[:, sl], in_=pq)
                pk = psMM.tile([128, 512], F32, tag="qkv", name="pk")[:RL, :]
                nc.tensor.matmul(
                    pk, _r(wr_sb[:, 1, :]), _r(rhT_s[:, sl]), start=True, stop=True
                )
                nc.vector.tensor_scalar_mul(out=rkT_s[:, sl], in0=pk, scalar1=RK_SCALE)

            # rkR[t, 4i+u] = rkT[u, 512t+i]  (reshape(4, 2048) of r_k)
            for tt in range(4):
                for u in range(RL):
                    nc.sync.dma_start(
                        out=_fview(rkR_s[tt : tt + 1, :], [[RL, 512]], u),
                        in_=rkT_s[u : u + 1, tt * 512 : (tt + 1) * 512],
                    )

            if debug:
                nc.sync.dma_start(out=dbg["rkR"][:, :], in_=rkR_s.bitcast(F32))
                nc.sync.dma_start(out=dbg["rqT"][:, :], in_=rqT_s.bitcast(F32))

            # q and v full [128, 8, ROWS]; weights loaded as [128, 512] half-rows
            for w_d, dest in ((t["wq_d"], qT_s), (t["wv_d"], vT_s)) if "qkv" in phases else ():
                for half in range(2):
                    wrows = [None] * 8
                    for ci in range(8):
                        wrow = wpool.tile([128, 512], F32R, tag="w", name="wrow")
                        nc.sync.dma_start(
                            out=wrow,
                            in_=w_d[
                                ci * 128 : (ci + 1) * 128,
                                half * 512 : (half + 1) * 512,
                            ].bitcast(F32R),
                        )
                        wrows[ci] = wrow
                    for col in range(4):
                        co = half * 4 + col
                        pm = psMM.tile([128, ROWS], F32, tag="qkv", name="pm")
                        for ci in range(8):
                            nc.tensor.matmul(
                                pm,
                                _r(wrows[ci][:, col * 128 : (col + 1) * 128]),
                                _r(xT_s[:, ci, :]),
                                start=(ci == 0),
                                stop=(ci == 7),
                            )
                        nc.vector.tensor_copy(
                            out=_fview(dest[:, :], [[8, ROWS]], co), in_=pm
                        )

            # k per-co streaming: each co slice feeds only KT tiles t%8==co
            for half in range(2 if "qkv" in phases else 0):
                wrows = [None] * 8
                for ci in range(8):
                    wrow = wpool.tile([128, 512], F32R, tag="w", name="wkrow")
                    nc.sync.dma_start(
                        out=wrow,
                        in_=t["wk_d"][
                            ci * 128 : (ci + 1) * 128, half * 512 : (half + 1) * 512
                        ].bitcast(F32R),
                    )
                    wrows[ci] = wrow
                for col in range(4):
                    co = half * 4 + col
                    pm = psMM.tile([128, ROWS], F32, tag="qkv", name="pmk")
                    for ci in range(8):
                        nc.tensor.matmul(
                            pm,
                            _r(wrows[ci][:, col * 128 : (col + 1) * 128]),
                            _r(xT_s[:, ci, :]),
                            start=(ci == 0),
                            stop=(ci == 7),
                        )
                    kco = kstream.tile([128, ROWS], F32, tag="kco", name="kco")
                    nc.vector.tensor_copy(out=kco, in_=pm)
                    for b in range(NBLK):
                        for tt in (co, co + 8):
                            # KT_t^T[mm, d] = k[256b + 2d + (t>=8), 128co + mm]
                            kt_view = _fview(
                                kco[:, :], [[2, 128]], 256 * b + (1 if tt >= 8 else 0)
                            )
                            p = psT2.tile([128, 128], F32, tag="pst2", name="pKT")
                            nc.tensor.transpose(p, kt_view, ident)
                            nc.vector.tensor_copy(out=KT_sb[b][:, tt, :], in_=p)

            # V tiles from vT_s
            for b in range(NBLK if "ktv" in phases else 0):
                for tt in range(16):
                    # V_t^T[e, 8a+j] = vT_s[e, j, 256b + 16t + a]
                    v_view = _fview(
                        vT_s[:, :], [[1, 128]], 8 * (256 * b + 16 * tt)
                    )
                    pv = psT2.tile([128, 128], F32, tag="pst2", name="pV")
                    nc.tensor.transpose(pv, v_view, ident)
                    nc.vector.tensor_copy(out=V_sb[b][:, tt, :], in_=pv)
            if debug:
                nc.sync.dma_start(out=dbg["qT"][:, :], in_=qT_s.bitcast(F32))
                nc.sync.dma_start(out=dbg["KT"][:, :, :], in_=KT_sb[0].bitcast(F32))
                nc.sync.dma_start(out=dbg["V"][:, :, :], in_=V_sb[0].bitcast(F32))
        kv_es.close()  # vT/k-stream dead once KT/V tiles exist

        # Wo preload: region reuses the kv pool space (freed at P3 end), so
        # this 4MB DMA overlaps the whole attention phase. Lives in qkv_es,
        # whose close moves to after the Wo phase to keep LIFO order.
        wopool = qkv_es.enter_context(tc.tile_pool(name="wotile", bufs=1))
        wo_s = wopool.tile([128, 8, D], F32R, name="wo_s")
        nc.sync.dma_start(
            out=wo_s,
            in_=t["wo_d"][:, :].rearrange("(j p) n -> p j n", p=128).bitcast(F32R),
        )
        g1b = bcast_load(wopool, t["g1_d"], "g1b")
        be1b = bcast_load(wopool, t["be1_d"], "be1b")

        def layer_norm(dest, pre, gb, bb, pool):
            """dest = LN(pre) * gb + bb ; pre is [128, 1024] SBUF.

            The gamma/beta apply is split by column half across gpsimd and
            DVE so the two chains run in parallel.
            """
            st = pool.tile([128, 2, 6], F32, tag="bnst", name="st")
            nc.vector.bn_stats(out=st[:, 0, :], in_=pre[:, 0:512])
            nc.vector.bn_stats(out=st[:, 1, :], in_=pre[:, 512:1024])
            mv = pool.tile([128, 2], F32, tag="bnmv", name="mv")
            nc.vector.bn_aggr(out=mv, in_=st)
            rstd = pool.tile([128, 1], F32, tag="rstd", name="rstd")
            nc.scalar.activation(
                out=rstd,
                in_=mv[:, 1:2],
                func=mybir.ActivationFunctionType.Sqrt,
                bias=eps_t,
            )
            nc.vector.reciprocal(out=rstd, in_=rstd)
            xn = pool.tile([128, D], F32, tag="xn", name="xn")
            nc.vector.tensor_scalar(
                out=xn,
                in0=pre,
                scalar1=mv[:, 0:1],
                scalar2=rstd,
                op0=mybir.AluOpType.subtract,
                op1=mybir.AluOpType.mult,
            )
            cs1, cs0 = slice(512, 1024), slice(0, 512)
            nc.gpsimd.tensor_mul(out=xn[:, cs1], in0=xn[:, cs1], in1=gb[:, cs1])
            nc.vector.tensor_mul(out=xn[:, cs0], in0=xn[:, cs0], in1=gb[:, cs0])
            nc.vector.tensor_add(out=dest[:, cs0], in0=xn[:, cs0], in1=bb[:, cs0])
            nc.vector.tensor_add(out=dest[:, cs1], in0=xn[:, cs1], in1=bb[:, cs1])

        # ---- phases 4+5 fused: attention with per-lh Wo + LN1 -----------
        # Wide [128,1024] score tiles halve the exp count (one Act exp per
        # key tile instead of two), cutting Act's fixed per-instruction
        # overhead — Act was the attention rate-limiter. PSUM: S 2x2 + C 2
        # + D 2 banks; Wo's accumulators borrow the D banks between lh
        # rounds (pD is dead after the reciprocal).
        with (
            tc.tile_pool(name="psS", bufs=2, space="PSUM") as psS,
            tc.tile_pool(name="psC", bufs=1, space="PSUM") as psC,
            tc.tile_pool(name="psD", bufs=1, space="PSUM") as psD,
            tc.tile_pool(name="epool", bufs=3) as epool,
            tc.tile_pool(name="inv", bufs=2) as invp,
            tc.tile_pool(name="lnp", bufs=2) as lnp,
        ):
            def wo_ln1(b, rc2):
                a = 2 * b + rc2  # core row-chunk index
                xrow = lnp.tile([128, D], F32, tag="xrow2", name="xrow2")
                nc.sync.dma_start(out=xrow, in_=x_d[a * 128 : (a + 1) * 128, :])
                pre = lnp.tile([128, D], F32, tag="pre", name="pre")
                for nchunk in range(2):
                    ph = psD.tile([128, 512], F32, tag="pd", name="ph")
                    for j in range(8):
                        ctx_view = _fview(CT_s[b][:, :], [[8, 128]], 1024 * rc2 + j)
                        nc.tensor.matmul(
                            ph,
                            _r(ctx_view),
                            _r(wo_s[:, j, nchunk * 512 : (nchunk + 1) * 512]),
                            start=(j == 0),
                            stop=(j == 7),
                        )
                    nc.vector.tensor_add(
                        out=pre[:, nchunk * 512 : (nchunk + 1) * 512],
                        in0=ph,
                        in1=xrow[:, nchunk * 512 : (nchunk + 1) * 512],
                    )
                layer_norm(h1_s[:, a, :], pre, g1b, be1b, lnp)

            for b in range(NBLK if "attn" in phases else 0):
                for lh in range(2):
                    pC = psC.tile([128, 1024], F32, tag="pc", name="pC")
                    pD = psD.tile([128, 1024], F32, tag="pd", name="pD")
                    # software pipeline: C/D for key-tile tt issue after the
                    # scores for tt+1, so the PE never waits on exp(tt)
                    e_tiles = {}
                    for tt in range(17):
                        if tt < 16:
                            e_t = epool.tile([128, 1024], F32R, tag="e", name="e_t")
                            e_tiles[tt] = e_t
                            pS = psS.tile([128, 1024], F32, tag="ps", name="pS")
                            for q in range(2):
                                lq = slice(q * 512, (q + 1) * 512)
                                # l = 1024*lh + 512*q + 8r + j
                                off = 8 * (256 * b + 128 * lh + 64 * q)
                                qt_view = qT_s[:, off : off + 512]
                                nc.tensor.matmul(
                                    pS[:, lq],
                                    _r(KT_sb[b][:, tt, :]),
                                    _r(qt_view),
                                    start=True,
                                    stop=False,
                                )
                                nc.tensor.matmul(
                                    pS[:, lq],
                                    _r(rkR_s[:, tt * 128 : (tt + 1) * 128]),
                                    _r(rqT_s[:, 1024 * lh + 512 * q :][:, :512]),
                                    start=False,
                                    stop=True,
                                )
                            nc.scalar.activation(
                                out=e_t,
                                in_=pS,
                                func=mybir.ActivationFunctionType.Exp,
                                scale=EXP_SCALE,
                            )
                            if debug and b == 0 and lh == 0 and tt == 0:
                                nc.sync.dma_start(
                                    out=dbg["E"][:, :], in_=e_t.bitcast(F32)
                                )
                        if tt >= 1:
                            tp = tt - 1
                            e_p = e_tiles.pop(tp)
                            for q in range(2):
                                lq = slice(q * 512, (q + 1) * 512)
                                nc.tensor.matmul(
                                    pC[:, lq],
                                    _r(V_sb[b][:, tp, :]),
                                    _r(e_p[:, lq]),
                                    start=(tp == 0),
                                    stop=(tp == 15),
                                )
                                nc.tensor.matmul(
                                    pD[:, lq],
                                    _r(ones_sb),
                                    _r(e_p[:, lq]),
                                    start=(tp == 0),
                                    stop=(tp == 15),
                                )
                    inv_t = invp.tile([128, 1024], F32, tag="inv", name="inv_t")
                    nc.vector.reciprocal(out=inv_t, in_=pD)
                    nc.vector.tensor_mul(
                        out=CT_s[b][:, 1024 * lh : 1024 * (lh + 1)],
                        in0=pC,
                        in1=inv_t,
                    )
                    if "wo" in phases:
                        wo_ln1(b, lh)
            if debug:
                nc.sync.dma_start(out=dbg["CT"][:, :], in_=CT_s[0].bitcast(F32))
                nc.sync.dma_start(out=dbg["h1"][:, :, :], in_=h1_s.bitcast(F32))
        qkv_es.close()  # qT/rel/KT/V/Wo dead after Wo+LN1
        ct_es.close()  # CT dead after Wo

        # FFN pools open before phase 6 so the first W1 half-group (and the
        # W2/relu regions) are live early; W1 hg=0 loads under the h1T phase.
        ffn2_es = ctx.enter_context(contextlib.ExitStack())
        w2pool = ffn2_es.enter_context(tc.tile_pool(name="w2tile", bufs=3))
        rkeep = ffn2_es.enter_context(tc.tile_pool(name="rkeep", bufs=32))
        w1_es = ctx.enter_context(contextlib.ExitStack())
        w1pool = w1_es.enter_context(tc.tile_pool(name="w1tile", bufs=14))
        rkeep_tiles = {}
        w1half = {}

        def w1_load(hg):
            tiles = [None] * 8
            for ci in range(8):
                wt = w1pool.tile([128, 512], F32R, tag="w1", name="w1t")
                nc.sync.dma_start(
                    out=wt,
                    in_=t["w1_d"][
                        ci * 128 : (ci + 1) * 128, hg * 512 : (hg + 1) * 512
                    ].bitcast(F32R),
                )
                tiles[ci] = wt
            w1half[hg] = tiles

        if "ffn1" in phases:
            w1_load(0)

        # ---- phase 6: h1T -----------------------------------------------
        with tc.tile_pool(name="psT3", bufs=2, space="PSUM") as psT3:
            for ct_i in range(8):
                for a in range(4):
                    p = psT3.tile([128, 128], F32R, tag="pst3", name="pH")
                    nc.tensor.transpose(
                        p, h1_s[:, a, ct_i * 128 : (ct_i + 1) * 128], ident_r
                    )
                    nc.vector.tensor_copy(
                        out=h1T_s[:, ct_i, a * 128 : (a + 1) * 128], in_=p
                    )

        # h1_s += b2 (broadcast) so FFN2's identity-inject seeds h1+b2.
        # Runs after the transposes read pristine h1; hidden under FFN1.
        b2b = bcast_load(singles, t["b2_d"], "b2b")
        for a in range(4):
            nc.vector.tensor_add(
                out=h1_s[:, a, :], in0=h1_s[:, a, :], in1=b2b
            )


        # ---- phase 7: FFN1 + relu (kept in SBUF as bf16) ----------------
        # W1 streams in 4-f half-groups ([128, 512] per ci) so buffers free
        # incrementally and the DMA stays ahead. All 32 relu tiles live in
        # SBUF as bf16 (4MB) — no DRAM bounce.
        with tc.tile_pool(name="psF1", bufs=4, space="PSUM") as psF1:
            w2g_pre = {}
            for f in range(32 if "ffn1" in phases else 0):
                hg, fl = f // 4, f % 4
                if fl == 0 and hg + 1 < 8:
                    w1_load(hg + 1)
                if f == 28:
                    # prefetch FFN2's first two weight groups while the last
                    # W1 half-groups stream
                    for g in range(2):
                        wg = w2pool.tile([128, 4, D], F32R, tag="w2", name="w2g")
                        nc.sync.dma_start(
                            out=wg,
                            in_=t["w2_d"][g * 512 : (g + 1) * 512, :]
                            .rearrange("(g p) c -> p g c", p=128)
                            .bitcast(F32R),
                        )
                        w2g_pre[g] = wg
                pm = psF1.tile([128, ROWS], F32, tag="psf1", name="pF")
                for ci in range(8):
                    nc.tensor.matmul(
                        pm,
                        _r(w1half[hg][ci][:, fl * 128 : (fl + 1) * 128]),
                        _r(h1T_s[:, ci, :]),
                        start=(ci == 0),
                        stop=(ci == 7),
                    )
                rt = rkeep.tile([128, ROWS], F32R, tag="rkeep", name="rk")
                rkeep_tiles[f] = rt
                nc.scalar.activation(
                    out=rt,
                    in_=pm,
                    func=mybir.ActivationFunctionType.Relu,
                    bias=b1t[:, f : f + 1],
                )
        w1_es.close()  # free W1 buffers before FFN2's pools allocate
        # ---- phase 8: FFN2 + residual + LN2 + store ---------------------
        # The h1+b2 residual is injected into each PSUM accumulator by an
        # identity-matmul at accumulation start (h1_s holds h1+b2 by then),
        # so the epilogue reads LN2 stats straight off PSUM. The last weight
        # group runs a-outer so accumulators finish staggered and the four
        # LN2 chains pipeline behind the remaining matmuls.
        with (
            tc.tile_pool(name="psF2", bufs=1, space="PSUM") as psF2,
            tc.tile_pool(name="ln2p", bufs=2) as ln2p,
            tc.tile_pool(name="outp", bufs=2) as outp,
        ):
            g2b = bcast_load(ln2p, t["g2_d"], "g2b")
            be2b = bcast_load(ln2p, t["be2_d"], "be2b")
            pacc = [
                psF2.tile([128, 512], F32, tag=f"psf2_{i}", name=f"psf2_{i}")
                for i in range(8)
            ]
            if "ffn2" in phases:
                for i in range(8):
                    a, cchunk = i // 2, i % 2
                    cs = slice(cchunk * 512, (cchunk + 1) * 512)
                    nc.tensor.matmul(
                        pacc[i], ident_r, h1_s[:, a, cs], start=True, stop=False
                    )

            def f2_mm(f, a, w2g):
                fl = f % 4
                for cchunk in range(2):
                    nc.tensor.matmul(
                        pacc[a * 2 + cchunk],
                        rkeep_tiles[f][:, a * 128 : (a + 1) * 128],
                        _r(w2g[:, fl, cchunk * 512 : (cchunk + 1) * 512]),
                        start=False,
                        stop=(f == 31),
                    )

            def ln2_epilogue(a):
                st = ln2p.tile([128, 2, 6], F32, tag="bnst", name="st2")
                nc.vector.bn_stats(out=st[:, 0, :], in_=pacc[a * 2])
                nc.vector.bn_stats(out=st[:, 1, :], in_=pacc[a * 2 + 1])
                mv = ln2p.tile([128, 2], F32, tag="bnmv", name="mv2")
                nc.vector.bn_aggr(out=mv, in_=st)
                rstd = ln2p.tile([128, 1], F32, tag="rstd", name="rstd2")
                nc.scalar.activation(
                    out=rstd,
                    in_=mv[:, 1:2],
                    func=mybir.ActivationFunctionType.Sqrt,
                    bias=eps_t,
                )
                nc.vector.reciprocal(out=rstd, in_=rstd)
                o_t = outp.tile([128, D], F32, tag="o", name="o_t")
                # normalize on DVE (PSUM access); gamma/beta apply split
                # across gpsimd (chunk 1) and DVE (chunk 0) to run in parallel
                for cchunk in range(2):
                    cs = slice(cchunk * 512, (cchunk + 1) * 512)
                    nc.vector.tensor_scalar(
                        out=o_t[:, cs],
                        in0=pacc[a * 2 + cchunk],
                        scalar1=mv[:, 0:1],
                        scalar2=rstd,
                        op0=mybir.AluOpType.subtract,
                        op1=mybir.AluOpType.mult,
                    )
                for cchunk, eng in ((1, nc.gpsimd), (0, nc.vector)):
                    cs = slice(cchunk * 512, (cchunk + 1) * 512)
                    eng.tensor_mul(out=o_t[:, cs], in0=o_t[:, cs], in1=g2b[:, cs])
                    eng.tensor_add(out=o_t[:, cs], in0=o_t[:, cs], in1=be2b[:, cs])
                nc.sync.dma_start(out=out_d[a * 128 : (a + 1) * 128, :], in_=o_t)

            for f in range(24 if "ffn2" in phases else 0):
                fg, fl = f // 4, f % 4
                if fl == 0:
                    if fg in w2g_pre:
                        w2g = w2g_pre[fg]
                    else:
                        w2g = w2pool.tile([128, 4, D], F32R, tag="w2", name="w2g")
                        nc.sync.dma_start(
                            out=w2g,
                            in_=t["w2_d"][fg * 512 : (fg + 1) * 512, :]
                            .rearrange("(g p) c -> p g c", p=128)
                            .bitcast(F32R),
                        )
                for a in range(4):
                    f2_mm(f, a, w2g)
            if "ffn2" in phases:
                # last two weight groups run a-outer: each row-chunk's
                # accumulators stop early and its LN2 chain overlaps the rest
                w2g_last = {}
                for fg in (6, 7):
                    wg = w2pool.tile([128, 4, D], F32R, tag="w2", name="w2g")
                    nc.sync.dma_start(
                        out=wg,
                        in_=t["w2_d"][fg * 512 : (fg + 1) * 512, :]
                        .rearrange("(g p) c -> p g c", p=128)
                        .bitcast(F32R),
                    )
                    w2g_last[fg] = wg
                for a in range(4):
                    for f in range(24, 32):
                        f2_mm(f, a, w2g_last[f // 4])
                    ln2_epilogue(a)


def _get_nc(debug=False):
    key = ("dbg" if debug else "main")
    if key not in _cache:
        _cache[key] = build_nc(debug)
    return _cache[key]


def kernel(**inputs):
    h = np.ascontiguousarray(np.asarray(inputs["h"], dtype=np.float32))
    rh = np.ascontiguousarray(np.asarray(inputs["rh"], dtype=np.float32))
    weights = {
        k: np.ascontiguousarray(np.asarray(inputs[k], dtype=np.float32))
        for k in (
            "Wq", "Wk", "Wv", "Wo", "Wrk", "Wrq",
            "W1", "b1", "W2", "b2", "g1", "be1", "g2", "be2",
        )
    }
    in_maps = []
    for c in range(8):
        b, r0 = c // 4, 512 * (c % 4)
        m = {"x": h[b, r0 : r0 + 512, :], "rh": rh[b]}
        m.update(weights)
        in_maps.append(m)

    nc = _get_nc()
    res = run_bass_kernel_spmd(nc, in_maps, core_ids=list(range(8)))
    out = np.empty((B, L, D), dtype=np.float32)
    for c in range(8):
        b, r0 = c // 4, 512 * (c % 4)
        out[b, r0 : r0 + 512, :] = res.results[c]["out"]
    return out



# revision 42
# speedup vs baseline: 1.0497x; 1.0207x over previous
"""Trainium2 Bass kernel for nn_GTLayer_84722524880938.

The reference uses .reshape (not transpose) for the attention head split,
which makes attention block-diagonal over 256-row blocks of the sequence:
output rows [256n, 256n+256) depend only on input rows [256n, 256n+256)
(plus the full-length relative-position bias, which is rank-4). The layer
therefore shards perfectly across 8 cores: core c takes 512 contiguous
rows (2 blocks) of batch c//4 and needs no collectives.

Per 256-row block (X = h[b, 256n:256n+256, :]):
  q = X@Wq; k = X@Wk; v = X@Wv            [256, 1024]
  Q = q.reshape(2048, 128); KT = k.reshape(128, 2048); V = v.reshape(2048, 128)
  S = Q@KT/sqrt(128) + (rh[b]@Wrq) @ (rh[b]@Wrk).reshape(4, 2048) / 2
  P = softmax(S, -1);  C = P@V            [2048, 128]
  h_sa = C.reshape(256, 1024) @ Wo
  h1 = LN(h_sa + X);  hf = relu(h1@W1 + b1)@W2 + b2;  out = LN(h1 + hf)

All matmuls run as float32r (full fp32 data, full-rate PE mode). Scores
are exponentiated without max-subtraction (|S| < ~14, far from fp32 exp
overflow). The softmax denominator comes from an extra ones-weight matmul
accumulated alongside P@V.
"""

import sys

sys.path.insert(0, "/opt/trn_rl_repo")

import math

import numpy as np

import concourse.bass as bass
import concourse.mybir as mybir
import concourse.tile as tile
from concourse.bass_utils import run_bass_kernel_spmd
from concourse.masks import make_identity

F32 = mybir.dt.float32
F32R = mybir.dt.float32r
BF16 = mybir.dt.bfloat16

D, FFN, NH, HD, RL = 1024, 4096, 8, 128, 4
B, L = 2, 2048
ROWS = 512  # rows per core
NBLK = 2  # 256-row attention blocks per core
EPS = 1e-5
EXP_SCALE = 1.0 / math.sqrt(HD)  # applied by ACT on scores
RK_SCALE = math.sqrt(HD) / 2.0  # folded into r_k so bias lands as bias/2

MAX_WAITS = 1  # this walrus build allows one semaphore wait per instruction

_cache = {}


def _fix_waits(nc):
    """Split >MAX_WAITS sync waits onto injected same-engine NoOps.

    Engines execute their stream in order, so hoisting excess waits onto
    NoOps placed immediately before the instruction preserves semantics.
    """
    ctr = 0
    for f in nc.m.functions:
        for blk in f.blocks:
            out = []
            changed = False
            for ins in blk.instructions:
                si = ins.sync_info
                waits = list(si.on_wait) if si is not None else []
                if len(waits) > MAX_WAITS:
                    changed = True
                    while len(waits) > MAX_WAITS:
                        chunk, waits = waits[:MAX_WAITS], waits[MAX_WAITS:]
                        ctr += 1
                        nop = mybir.InstNoOp(
                            name=f"waitfix-nop-{ctr}",
                            ins=[],
                            outs=[],
                            sync_info=mybir.SyncInfo(on_wait=chunk, on_update=[]),
                        )
                        nop.engine = ins.engine
                        out.append(nop)
                    ins.sync_info = mybir.SyncInfo(
                        on_wait=waits, on_update=list(si.on_update)
                    )
                out.append(ins)
            if changed:
                blk.instructions = out
    return nc


def _r(ap):
    return ap.bitcast(F32R)


def _fview(base, free_dims, extra_off=0):
    """Rebuild an AP keeping the partition dim, with custom free dims/offset."""
    return bass.AP(
        tensor=base.tensor,
        offset=base.offset + extra_off,
        ap=[list(base.ap[0])] + [list(d) for d in free_dims],
    )


def build_nc(debug=False, repeat=1, phases=None):
    nc = bass.Bass(target_bir_lowering=False)

    x_d = nc.dram_tensor("x", [ROWS, D], F32, kind="ExternalInput")
    rh_d = nc.dram_tensor("rh", [L, RL], F32, kind="ExternalInput")
    wq_d = nc.dram_tensor("Wq", [D, D], F32, kind="ExternalInput")
    wk_d = nc.dram_tensor("Wk", [D, D], F32, kind="ExternalInput")
    wv_d = nc.dram_tensor("Wv", [D, D], F32, kind="ExternalInput")
    wo_d = nc.dram_tensor("Wo", [D, D], F32, kind="ExternalInput")
    wrk_d = nc.dram_tensor("Wrk", [RL, RL], F32, kind="ExternalInput")
    wrq_d = nc.dram_tensor("Wrq", [RL, RL], F32, kind="ExternalInput")
    w1_d = nc.dram_tensor("W1", [D, FFN], F32, kind="ExternalInput")
    b1_d = nc.dram_tensor("b1", [FFN], F32, kind="ExternalInput")
    w2_d = nc.dram_tensor("W2", [FFN, D], F32, kind="ExternalInput")
    b2_d = nc.dram_tensor("b2", [D], F32, kind="ExternalInput")
    g1_d = nc.dram_tensor("g1", [D], F32, kind="ExternalInput")
    be1_d = nc.dram_tensor("be1", [D], F32, kind="ExternalInput")
    g2_d = nc.dram_tensor("g2", [D], F32, kind="ExternalInput")
    be2_d = nc.dram_tensor("be2", [D], F32, kind="ExternalInput")
    out_d = nc.dram_tensor("out", [ROWS, D], F32, kind="ExternalOutput")

    dbg = {}
    if debug:
        dbg["qT"] = nc.dram_tensor("dbg_qT", [128, NH * ROWS], F32, kind="ExternalOutput")
        dbg["KT"] = nc.dram_tensor("dbg_KT", [128, 16, 128], F32, kind="ExternalOutput")
        dbg["V"] = nc.dram_tensor("dbg_V", [128, 16, 128], F32, kind="ExternalOutput")
        dbg["rkR"] = nc.dram_tensor("dbg_rkR", [RL, L], F32, kind="ExternalOutput")
        dbg["rqT"] = nc.dram_tensor("dbg_rqT", [RL, L], F32, kind="ExternalOutput")
        dbg["E"] = nc.dram_tensor("dbg_E", [128, 1024], F32, kind="ExternalOutput")
        dbg["CT"] = nc.dram_tensor("dbg_CT", [128, L], F32, kind="ExternalOutput")
        dbg["h1"] = nc.dram_tensor("dbg_h1", [128, 4, D], F32, kind="ExternalOutput")
        dbg["relu"] = nc.dram_tensor("dbg_relu", [128, ROWS], F32, kind="ExternalOutput")

    ph = phases
    with tile.TileContext(nc, pool_alloc_mode="stack") as tc:
        for _rep in range(repeat):
            _body(nc, tc, locals())

    _fix_waits(nc)
    return nc


def _body(nc, tc, t):
    phases = t["ph"] or {"qkv", "ktv", "attn", "wo", "ffn1", "ffn2"}
    debug = t["debug"]
    dbg = t["dbg"]
    x_d, rh_d, out_d = t["x_d"], t["rh_d"], t["out_d"]

    import contextlib

    ctx = contextlib.ExitStack()
    with ctx:
        # ---- pools ordered by lifetime (longest-lived first) ------------
        singles = ctx.enter_context(tc.tile_pool(name="singles", bufs=1))
        h1T_es = ctx.enter_context(contextlib.ExitStack())
        ct_es = h1T_es.enter_context(contextlib.ExitStack())
        qkv_es = ct_es.enter_context(contextlib.ExitStack())
        kv_es = qkv_es.enter_context(contextlib.ExitStack())

        ident = singles.tile([128, 128], F32)
        make_identity(nc, ident)
        ident_r = singles.tile([128, 128], F32R, name="ident_r")
        nc.vector.tensor_copy(out=ident_r, in_=ident)
        ones_f = singles.tile([128, 128], F32, name="ones_f")
        nc.vector.memset(ones_f, 1.0)
        ones_sb = singles.tile([128, 128], F32R, name="ones_sb")
        nc.vector.tensor_copy(out=ones_sb, in_=ones_f)
        eps_t = singles.tile([128, 1], F32)
        nc.vector.memset(eps_t, EPS)

        def bcast_load(pool, dram, name):
            tl = pool.tile([128, D], F32, name=name, tag=name)
            src = bass.AP(tensor=dram, offset=0, ap=[[0, 128], [1, D]])
            nc.sync.dma_start(out=tl, in_=src)
            return tl

        b1t = singles.tile([128, FFN // 128], F32)
        nc.sync.dma_start(
            out=b1t,
            in_=bass.AP(tensor=t["b1_d"], offset=0, ap=[[1, 128], [128, FFN // 128]]),
        )
        h1_s = singles.tile([128, 4, D], F32R, name="h1_s")  # written after LN1


        h1T_pool = h1T_es.enter_context(tc.tile_pool(name="h1T", bufs=1))
        h1T_s = h1T_pool.tile([128, 8, ROWS], F32R, name="h1T_s")
        # W1 prefetch region: reserved from program start (never aliases the
        # attention pools) so the first half-group can DMA during attention
        w1pre = h1T_es.enter_context(tc.tile_pool(name="w1pre", bufs=4))

        ct_pool = ct_es.enter_context(tc.tile_pool(name="ct", bufs=1))
        CT_s = [ct_pool.tile([128, L], F32R, name=f"CTb{b}", tag=f"CTb{b}") for b in range(NBLK)]

        qT_pool = qkv_es.enter_context(tc.tile_pool(name="qT", bufs=1))
        qT_s = qT_pool.tile([128, NH * ROWS], F32R, name="qT_s")
        rqT_s = qT_pool.tile([RL, L], F32R, name="rqT_s")
        rkR_s = qT_pool.tile([RL, L], F32R, name="rkR_s")
        ktv_pool = qkv_es.enter_context(tc.tile_pool(name="ktv", bufs=1))
        KT_sb = [ktv_pool.tile([128, 16, 128], F32R, name=f"KTb{b}", tag=f"KTb{b}") for b in range(NBLK)]
        V_sb = [ktv_pool.tile([128, 16, 128], F32R, name=f"Vb{b}", tag=f"Vb{b}") for b in range(NBLK)]

        kv_pool = kv_es.enter_context(tc.tile_pool(name="kv", bufs=1))
        vT_s = kv_pool.tile([128, NH * ROWS], F32, name="vT_s")
        kstream = kv_es.enter_context(tc.tile_pool(name="kstream", bufs=3))

        # ---- phase 1+2: XT, rel-bias, q/k/v; then KT/V tiles ------------
        with (
            tc.tile_pool(name="xt", bufs=1) as xt_pool,
            tc.tile_pool(name="psT", bufs=2, space="PSUM") as psT,
            tc.tile_pool(name="psMM", bufs=4, space="PSUM") as psMM,
            tc.tile_pool(name="psT2", bufs=2, space="PSUM") as psT2,
            tc.tile_pool(name="wtile", bufs=9) as wpool,
            tc.tile_pool(name="cpy", bufs=3) as cpy,
        ):
            xT_s = xt_pool.tile([128, 8, ROWS], F32R, name="xT_s")
            for rc in range(4):
                xrow = cpy.tile([128, D], F32, tag="xrow", name="xrow")
                nc.sync.dma_start(out=xrow, in_=x_d[rc * 128 : (rc + 1) * 128, :])
                for ct_i in range(8):
                    p = psT.tile([128, 128], F32, tag="pst", name="pT")
                    nc.tensor.transpose(p, xrow[:, ct_i * 128 : (ct_i + 1) * 128], ident)
                    nc.vector.tensor_copy(
                        out=xT_s[:, ct_i, rc * 128 : (rc + 1) * 128], in_=p
                    )

            # rhT [4, 2048] via 16 PE transposes of [128, 4] row tiles
            rh_sb = cpy.tile([128, 16, RL], F32, tag="rh", name="rh_sb")
            nc.sync.dma_start(
                out=rh_sb, in_=rh_d[:, :].rearrange("(a p) u -> p a u", p=128)
            )
            rhT_s = xt_pool.tile([RL, L], F32R, name="rhT_s")
            for a in range(16):
                p = psT.tile([128, 128], F32, tag="pst", name="pT2")
                nc.tensor.transpose(p[:RL, :], rh_sb[:, a, :], ident)
                nc.vector.tensor_copy(
                    out=rhT_s[:, a * 128 : (a + 1) * 128], in_=p[:RL, :]
                )

            # r_qT / r_kT: [4, 2048] = Wr.T @ rh.T
            wr_sb = cpy.tile([RL, 2, RL], F32R, tag="wr", name="wr_sb")
            nc.sync.dma_start(out=wr_sb[:, 0, :], in_=t["wrq_d"][:, :].bitcast(F32R))
            nc.sync.dma_start(out=wr_sb[:, 1, :], in_=t["wrk_d"][:, :].bitcast(F32R))
            rkT_s = xt_pool.tile([RL, L], F32R, name="rkT_s")
            for half in range(4):
                sl = slice(half * 512, (half + 1) * 512)
                pq = psMM.tile([128, 512], F32, tag="qkv", name="pq")[:RL, :]
                nc.tensor.matmul(
                    pq, _r(wr_sb[:, 0, :]), _r(rhT_s[:, sl]), start=True, stop=True
                )
                nc.vector.tensor_copy(out=rqT_s[:, sl], in_=pq)
                pk = psMM.tile([128, 512], F32, tag="qkv", name="pk")[:RL, :]
                nc.tensor.matmul(
                    pk, _r(wr_sb[:, 1, :]), _r(rhT_s[:, sl]), start=True, stop=True
                )
                nc.vector.tensor_scalar_mul(out=rkT_s[:, sl], in0=pk, scalar1=RK_SCALE)

            # rkR[t, 4i+u] = rkT[u, 512t+i]  (reshape(4, 2048) of r_k)
            for tt in range(4):
                for u in range(RL):
                    nc.sync.dma_start(
                        out=_fview(rkR_s[tt : tt + 1, :], [[RL, 512]], u),
                        in_=rkT_s[u : u + 1, tt * 512 : (tt + 1) * 512],
                    )

            if debug:
                nc.sync.dma_start(out=dbg["rkR"][:, :], in_=rkR_s.bitcast(F32))
                nc.sync.dma_start(out=dbg["rqT"][:, :], in_=rqT_s.bitcast(F32))

            # q and v full [128, 8, ROWS]; weights loaded as [128, 512] half-rows
            for w_d, dest in ((t["wq_d"], qT_s), (t["wv_d"], vT_s)) if "qkv" in phases else ():
                for half in range(2):
                    wrows = [None] * 8
                    for ci in range(8):
                        wrow = wpool.tile([128, 512], F32R, tag="w", name="wrow")
                        nc.sync.dma_start(
                            out=wrow,
                            in_=w_d[
                                ci * 128 : (ci + 1) * 128,
                                half * 512 : (half + 1) * 512,
                            ].bitcast(F32R),
                        )
                        wrows[ci] = wrow
                    for col in range(4):
                        co = half * 4 + col
                        pm = psMM.tile([128, ROWS], F32, tag="qkv", name="pm")
                        for ci in range(8):
                            nc.tensor.matmul(
                                pm,
                                _r(wrows[ci][:, col * 128 : (col + 1) * 128]),
                                _r(xT_s[:, ci, :]),
                                start=(ci == 0),
                                stop=(ci == 7),
                            )
                        nc.vector.tensor_copy(
                            out=_fview(dest[:, :], [[8, ROWS]], co), in_=pm
                        )

            # k per-co streaming: each co slice feeds only KT tiles t%8==co
            for half in range(2 if "qkv" in phases else 0):
                wrows = [None] * 8
                for ci in range(8):
                    wrow = wpool.tile([128, 512], F32R, tag="w", name="wkrow")
                    nc.sync.dma_start(
                        out=wrow,
                        in_=t["wk_d"][
                            ci * 128 : (ci + 1) * 128, half * 512 : (half + 1) * 512
                        ].bitcast(F32R),
                    )
                    wrows[ci] = wrow
                for col in range(4):
                    co = half * 4 + col
                    pm = psMM.tile([128, ROWS], F32, tag="qkv", name="pmk")
                    for ci in range(8):
                        nc.tensor.matmul(
                            pm,
                            _r(wrows[ci][:, col * 128 : (col + 1) * 128]),
                            _r(xT_s[:, ci, :]),
                            start=(ci == 0),
                            stop=(ci == 7),
                        )
                    kco = kstream.tile([128, ROWS], F32, tag="kco", name="kco")
                    nc.vector.tensor_copy(out=kco, in_=pm)
                    for b in range(NBLK):
                        for tt in (co, co + 8):
                            # KT_t^T[mm, d] = k[256b + 2d + (t>=8), 128co + mm]
                            kt_view = _fview(
                                kco[:, :], [[2, 128]], 256 * b + (1 if tt >= 8 else 0)
                            )
                            p = psT2.tile([128, 128], F32, tag="pst2", name="pKT")
                            nc.tensor.transpose(p, kt_view, ident)
                            nc.vector.tensor_copy(out=KT_sb[b][:, tt, :], in_=p)

            # V tiles from vT_s
            for b in range(NBLK if "ktv" in phases else 0):
                for tt in range(16):
                    # V_t^T[e, 8a+j] = vT_s[e, j, 256b + 16t + a]
                    v_view = _fview(
                        vT_s[:, :], [[1, 128]], 8 * (256 * b + 16 * tt)
                    )
                    pv = psT2.tile([128, 128], F32, tag="pst2", name="pV")
                    nc.tensor.transpose(pv, v_view, ident)
                    nc.vector.tensor_copy(out=V_sb[b][:, tt, :], in_=pv)
            if debug:
                nc.sync.dma_start(out=dbg["qT"][:, :], in_=qT_s.bitcast(F32))
                nc.sync.dma_start(out=dbg["KT"][:, :, :], in_=KT_sb[0].bitcast(F32))
                nc.sync.dma_start(out=dbg["V"][:, :, :], in_=V_sb[0].bitcast(F32))
        kv_es.close()  # vT/k-stream dead once KT/V tiles exist

        # Wo preload: region reuses the kv pool space (freed at P3 end), so
        # this 4MB DMA overlaps the whole attention phase. Lives in qkv_es,
        # whose close moves to after the Wo phase to keep LIFO order.
        wopool = qkv_es.enter_context(tc.tile_pool(name="wotile", bufs=1))
        wo_s = wopool.tile([128, 8, D], F32R, name="wo_s")
        nc.sync.dma_start(
            out=wo_s,
            in_=t["wo_d"][:, :].rearrange("(j p) n -> p j n", p=128).bitcast(F32R),
        )
        g1b = bcast_load(wopool, t["g1_d"], "g1b")
        be1b = bcast_load(wopool, t["be1_d"], "be1b")
        # W1 first half-group: DMAs issue here so they transfer during
        # attention into the reserved w1pre region
        w1half = {}
        if "ffn1" in phases:
            tiles = [None] * 8
            for ci in range(4):
                wt = w1pre.tile([128, 512], F32R, tag="w1p", name="w1p")
                nc.sync.dma_start(
                    out=wt,
                    in_=t["w1_d"][ci * 128 : (ci + 1) * 128, 0:512].bitcast(F32R),
                )
                tiles[ci] = wt
            w1half[0] = tiles
        # residual x rows for Wo+LN1: loaded up front so no SP-queue stalls
        xrow_pool = qkv_es.enter_context(tc.tile_pool(name="xrows", bufs=1))
        xrow_t = xrow_pool.tile([128, 4, D], F32, name="xrow_t")
        nc.sync.dma_start(
            out=xrow_t, in_=x_d[:, :].rearrange("(a p) n -> p a n", p=128)
        )

        def layer_norm(dest, pre, gb, bb, pool):
            """dest = LN(pre) * gb + bb ; pre is [128, 1024] SBUF.

            The gamma/beta apply is split by column half across gpsimd and
            DVE so the two chains run in parallel.
            """
            st = pool.tile([128, 2, 6], F32, tag="bnst", name="st")
            nc.vector.bn_stats(out=st[:, 0, :], in_=pre[:, 0:512])
            nc.vector.bn_stats(out=st[:, 1, :], in_=pre[:, 512:1024])
            mv = pool.tile([128, 2], F32, tag="bnmv", name="mv")
            nc.vector.bn_aggr(out=mv, in_=st)
            rstd = pool.tile([128, 1], F32, tag="rstd", name="rstd")
            nc.scalar.activation(
                out=rstd,
                in_=mv[:, 1:2],
                func=mybir.ActivationFunctionType.Sqrt,
                bias=eps_t,
            )
            nc.vector.reciprocal(out=rstd, in_=rstd)
            # normalize+apply runs in place over pre, halves split across
            # gpsimd and DVE
            cs1, cs0 = slice(512, 1024), slice(0, 512)
            for cs, eng in ((cs1, nc.gpsimd), (cs0, nc.vector)):
                eng.tensor_scalar(
                    out=pre[:, cs],
                    in0=pre[:, cs],
                    scalar1=mv[:, 0:1],
                    scalar2=rstd,
                    op0=mybir.AluOpType.subtract,
                    op1=mybir.AluOpType.mult,
                )
            nc.gpsimd.tensor_mul(out=pre[:, cs1], in0=pre[:, cs1], in1=gb[:, cs1])
            nc.vector.tensor_mul(out=pre[:, cs0], in0=pre[:, cs0], in1=gb[:, cs0])
            nc.vector.tensor_add(out=dest[:, cs0], in0=pre[:, cs0], in1=bb[:, cs0])
            nc.vector.tensor_add(out=dest[:, cs1], in0=pre[:, cs1], in1=bb[:, cs1])

        # ---- phases 4+5 fused: attention with per-lh Wo + LN1 -----------
        # Wide [128,1024] score tiles halve the exp count (one Act exp per
        # key tile instead of two), cutting Act's fixed per-instruction
        # overhead — Act was the attention rate-limiter. PSUM: S 2x2 + C 2
        # + D 2 banks; Wo's accumulators borrow the D banks between lh
        # rounds (pD is dead after the reciprocal).
        with (
            tc.tile_pool(name="psS", bufs=2, space="PSUM") as psS,
            tc.tile_pool(name="psC", bufs=1, space="PSUM") as psC,
            tc.tile_pool(name="psD", bufs=1, space="PSUM") as psD,
            tc.tile_pool(name="epool", bufs=3) as epool,
            tc.tile_pool(name="inv", bufs=1) as invp,
            tc.tile_pool(name="lnp", bufs=2) as lnp,
        ):
            def wo_ln1(b, rc2):
                a = 2 * b + rc2  # core row-chunk index
                xrow = xrow_t[:, a, :]
                pre = lnp.tile([128, D], F32, tag="pre", name="pre")
                for nchunk in range(2):
                    ph = psD.tile([128, 512], F32, tag="pd", name="ph")
                    for j in range(8):
                        ctx_view = _fview(CT_s[b][:, :], [[8, 128]], 1024 * rc2 + j)
                        nc.tensor.matmul(
                            ph,
                            _r(ctx_view),
                            _r(wo_s[:, j, nchunk * 512 : (nchunk + 1) * 512]),
                            start=(j == 0),
                            stop=(j == 7),
                        )
                    nc.vector.tensor_add(
                        out=pre[:, nchunk * 512 : (nchunk + 1) * 512],
                        in0=ph,
                        in1=xrow[:, nchunk * 512 : (nchunk + 1) * 512],
                    )
                layer_norm(h1_s[:, a, :], pre, g1b, be1b, lnp)

            for b in range(NBLK if "attn" in phases else 0):
                for lh in range(2):
                    pC = psC.tile([128, 1024], F32, tag="pc", name="pC")
                    pD = psD.tile([128, 1024], F32, tag="pd", name="pD")
                    # software pipeline: C/D for key-tile tt issue after the
                    # scores for tt+1, so the PE never waits on exp(tt)
                    e_tiles = {}
                    for tt in range(17):
                        if tt < 16:
                            e_t = epool.tile([128, 1024], F32R, tag="e", name="e_t")
                            e_tiles[tt] = e_t
                            pS = psS.tile([128, 1024], F32, tag="ps", name="pS")
                            for q in range(2):
                                lq = slice(q * 512, (q + 1) * 512)
                                # l = 1024*lh + 512*q + 8r + j
                                off = 8 * (256 * b + 128 * lh + 64 * q)
                                qt_view = qT_s[:, off : off + 512]
                                nc.tensor.matmul(
                                    pS[:, lq],
                                    _r(KT_sb[b][:, tt, :]),
                                    _r(qt_view),
                                    start=True,
                                    stop=False,
                                )
                                nc.tensor.matmul(
                                    pS[:, lq],
                                    _r(rkR_s[:, tt * 128 : (tt + 1) * 128]),
                                    _r(rqT_s[:, 1024 * lh + 512 * q :][:, :512]),
                                    start=False,
                                    stop=True,
                                )
                            nc.scalar.activation(
                                out=e_t,
                                in_=pS,
                                func=mybir.ActivationFunctionType.Exp,
                                scale=EXP_SCALE,
                            )
                            if debug and b == 0 and lh == 0 and tt == 0:
                                nc.sync.dma_start(
                                    out=dbg["E"][:, :], in_=e_t.bitcast(F32)
                                )
                        if tt >= 1:
                            tp = tt - 1
                            e_p = e_tiles.pop(tp)
                            for q in range(2):
                                lq = slice(q * 512, (q + 1) * 512)
                                nc.tensor.matmul(
                                    pC[:, lq],
                                    _r(V_sb[b][:, tp, :]),
                                    _r(e_p[:, lq]),
                                    start=(tp == 0),
                                    stop=(tp == 15),
                                )
                                nc.tensor.matmul(
                                    pD[:, lq],
                                    _r(ones_sb),
                                    _r(e_p[:, lq]),
                                    start=(tp == 0),
                                    stop=(tp == 15),
                                )
                    inv_t = invp.tile([128, 1024], F32, tag="inv", name="inv_t")
                    for q in range(2):
                        lq = slice(q * 512, (q + 1) * 512)
                        nc.vector.reciprocal(out=inv_t[:, lq], in_=pD[:, lq])
                        nc.vector.tensor_mul(
                            out=CT_s[b][:, 1024 * lh + q * 512 :][:, :512],
                            in0=pC[:, lq],
                            in1=inv_t[:, lq],
                        )
                    if "wo" in phases:
                        wo_ln1(b, lh)
            if debug:
                nc.sync.dma_start(out=dbg["CT"][:, :], in_=CT_s[0].bitcast(F32))
                nc.sync.dma_start(out=dbg["h1"][:, :, :], in_=h1_s.bitcast(F32))
        qkv_es.close()  # qT/rel/KT/V/Wo dead after Wo+LN1
        ct_es.close()  # CT dead after Wo

        # FFN pools open before phase 6 so the first W1 half-group (and the
        # W2/relu regions) are live early; W1 hg=0 loads under the h1T phase.
        ffn2_es = ctx.enter_context(contextlib.ExitStack())
        w2pool = ffn2_es.enter_context(tc.tile_pool(name="w2tile", bufs=3))
        rkeep = ffn2_es.enter_context(tc.tile_pool(name="rkeep", bufs=32))
        w1_es = ctx.enter_context(contextlib.ExitStack())
        w1pool = w1_es.enter_context(tc.tile_pool(name="w1tile", bufs=14))
        rkeep_tiles = {}

        def w1_load(hg):
            tiles = [None] * 8
            for ci in range(8):
                wt = w1pool.tile([128, 512], F32R, tag="w1", name="w1t")
                nc.sync.dma_start(
                    out=wt,
                    in_=t["w1_d"][
                        ci * 128 : (ci + 1) * 128, hg * 512 : (hg + 1) * 512
                    ].bitcast(F32R),
                )
                tiles[ci] = wt
            w1half[hg] = tiles

        if "ffn1" in phases:
            # rest of hg0 (ci 4..7) plus hg1 via the streaming pool
            for ci in range(4, 8):
                wt = w1pool.tile([128, 512], F32R, tag="w1", name="w1t")
                nc.sync.dma_start(
                    out=wt,
                    in_=t["w1_d"][ci * 128 : (ci + 1) * 128, 0:512].bitcast(F32R),
                )
                w1half[0][ci] = wt
            w1_load(1)

        # ---- phase 6: h1T -----------------------------------------------
        # ci-outer so FFN1's first accumulation chain can start after the
        # first few transposes; copies alternate DVE/gpsimd
        with tc.tile_pool(name="psT3", bufs=4, space="PSUM") as psT3:
            for ct_i in range(8):
                for a in range(4):
                    p = psT3.tile([128, 128], F32R, tag="pst3", name="pH")
                    nc.tensor.transpose(
                        p, h1_s[:, a, ct_i * 128 : (ct_i + 1) * 128], ident_r
                    )
                    if a % 2:
                        nc.vector.tensor_copy(
                            out=h1T_s[:, ct_i, a * 128 : (a + 1) * 128], in_=p
                        )
                    else:
                        nc.scalar.activation(
                            out=h1T_s[:, ct_i, a * 128 : (a + 1) * 128],
                            in_=p,
                            func=mybir.ActivationFunctionType.Copy,
                        )

        # h1_s += b2 (broadcast) so FFN2's identity-inject seeds h1+b2.
        # Runs after the transposes read pristine h1; hidden under FFN1.
        b2b = rkeep.tile([128, D], F32, name="b2b", tag="b2b", bufs=1)
        nc.sync.dma_start(
            out=b2b, in_=bass.AP(tensor=t["b2_d"], offset=0, ap=[[0, 128], [1, D]])
        )
        for a in range(4):
            nc.vector.tensor_add(
                out=h1_s[:, a, :], in0=h1_s[:, a, :], in1=b2b
            )


        # ---- phase 7: FFN1 + relu (kept in SBUF as bf16) ----------------
        # W1 streams in 4-f half-groups ([128, 512] per ci) so buffers free
        # incrementally and the DMA stays ahead. All 32 relu tiles live in
        # SBUF as bf16 (4MB) — no DRAM bounce.
        with tc.tile_pool(name="psF1", bufs=4, space="PSUM") as psF1:
            w2g_pre = {}
            for f in range(32 if "ffn1" in phases else 0):
                hg, fl = f // 4, f % 4
                if fl == 0 and hg + 2 < 8:
                    w1_load(hg + 2)
                if f == 28:
                    # prefetch FFN2's first two weight groups while the last
                    # W1 half-groups stream
                    for g in range(2):
                        wg = w2pool.tile([128, 4, D], F32R, tag="w2", name="w2g")
                        nc.sync.dma_start(
                            out=wg,
                            in_=t["w2_d"][g * 512 : (g + 1) * 512, :]
                            .rearrange("(g p) c -> p g c", p=128)
                            .bitcast(F32R),
                        )
                        w2g_pre[g] = wg
                pm = psF1.tile([128, ROWS], F32, tag="psf1", name="pF")
                for ci in range(8):
                    nc.tensor.matmul(
                        pm,
                        _r(w1half[hg][ci][:, fl * 128 : (fl + 1) * 128]),
                        _r(h1T_s[:, ci, :]),
                        start=(ci == 0),
                        stop=(ci == 7),
                    )
                rt = rkeep.tile([128, ROWS], F32R, tag="rkeep", name="rk")
                rkeep_tiles[f] = rt
                nc.scalar.activation(
                    out=rt,
                    in_=pm,
                    func=mybir.ActivationFunctionType.Relu,
                    bias=b1t[:, f : f + 1],
                )
        w1_es.close()  # free W1 buffers before FFN2's pools allocate
        # ---- phase 8: FFN2 + residual + LN2 + store ---------------------
        # The h1+b2 residual is injected into each PSUM accumulator by an
        # identity-matmul at accumulation start (h1_s holds h1+b2 by then),
        # so the epilogue reads LN2 stats straight off PSUM. The last weight
        # group runs a-outer so accumulators finish staggered and the four
        # LN2 chains pipeline behind the remaining matmuls.
        with (
            tc.tile_pool(name="psF2", bufs=1, space="PSUM") as psF2,
            tc.tile_pool(name="ln2p", bufs=2) as ln2p,
            tc.tile_pool(name="outp", bufs=2) as outp,
        ):
            g2b = bcast_load(ln2p, t["g2_d"], "g2b")
            be2b = bcast_load(ln2p, t["be2_d"], "be2b")
            pacc = [
                psF2.tile([128, 512], F32, tag=f"psf2_{i}", name=f"psf2_{i}")
                for i in range(8)
            ]
            if "ffn2" in phases:
                for i in range(8):
                    a, cchunk = i // 2, i % 2
                    cs = slice(cchunk * 512, (cchunk + 1) * 512)
                    nc.tensor.matmul(
                        pacc[i], ident_r, h1_s[:, a, cs], start=True, stop=False
                    )

            def f2_mm(f, a, w2g):
                fl = f % 4
                for cchunk in range(2):
                    nc.tensor.matmul(
                        pacc[a * 2 + cchunk],
                        rkeep_tiles[f][:, a * 128 : (a + 1) * 128],
                        _r(w2g[:, fl, cchunk * 512 : (cchunk + 1) * 512]),
                        start=False,
                        stop=(f == 31),
                    )

            def ln2_epilogue(a):
                st = ln2p.tile([128, 2, 6], F32, tag="bnst", name="st2")
                nc.vector.bn_stats(out=st[:, 0, :], in_=pacc[a * 2])
                nc.vector.bn_stats(out=st[:, 1, :], in_=pacc[a * 2 + 1])
                mv = ln2p.tile([128, 2], F32, tag="bnmv", name="mv2")
                nc.vector.bn_aggr(out=mv, in_=st)
                rstd = ln2p.tile([128, 1], F32, tag="rstd", name="rstd2")
                nc.scalar.activation(
                    out=rstd,
                    in_=mv[:, 1:2],
                    func=mybir.ActivationFunctionType.Sqrt,
                    bias=eps_t,
                )
                nc.vector.reciprocal(out=rstd, in_=rstd)
                o_t = outp.tile([128, D], F32, tag="o", name="o_t")
                # normalize on DVE (PSUM access); gamma/beta apply split
                # across gpsimd (chunk 1) and DVE (chunk 0) to run in parallel
                for cchunk in range(2):
                    cs = slice(cchunk * 512, (cchunk + 1) * 512)
                    nc.vector.tensor_scalar(
                        out=o_t[:, cs],
                        in0=pacc[a * 2 + cchunk],
                        scalar1=mv[:, 0:1],
                        scalar2=rstd,
                        op0=mybir.AluOpType.subtract,
                        op1=mybir.AluOpType.mult,
                    )
                for cchunk, eng in ((1, nc.gpsimd), (0, nc.vector)):
                    cs = slice(cchunk * 512, (cchunk + 1) * 512)
                    eng.tensor_mul(out=o_t[:, cs], in0=o_t[:, cs], in1=g2b[:, cs])
                    eng.tensor_add(out=o_t[:, cs], in0=o_t[:, cs], in1=be2b[:, cs])
                nc.sync.dma_start(out=out_d[a * 128 : (a + 1) * 128, :], in_=o_t)

            for f in range(24 if "ffn2" in phases else 0):
                fg, fl = f // 4, f % 4
                if fl == 0:
                    if fg in w2g_pre:
                        w2g = w2g_pre[fg]
                    else:
                        w2g = w2pool.tile([128, 4, D], F32R, tag="w2", name="w2g")
                        nc.sync.dma_start(
                            out=w2g,
                            in_=t["w2_d"][fg * 512 : (fg + 1) * 512, :]
                            .rearrange("(g p) c -> p g c", p=128)
                            .bitcast(F32R),
                        )
                for a in range(4):
                    f2_mm(f, a, w2g)
            if "ffn2" in phases:
                # last two weight groups run a-outer: each row-chunk's
                # accumulators stop early and its LN2 chain overlaps the rest
                w2g_last = {}
                for fg in (6, 7):
                    wg = w2pool.tile([128, 4, D], F32R, tag="w2", name="w2g")
                    nc.sync.dma_start(
                        out=wg,
                        in_=t["w2_d"][fg * 512 : (fg + 1) * 512, :]
                        .rearrange("(g p) c -> p g c", p=128)
                        .bitcast(F32R),
                    )
                    w2g_last[fg] = wg
                for a in range(4):
                    for f in range(24, 32):
                        f2_mm(f, a, w2g_last[f // 4])
                    ln2_epilogue(a)


def _get_nc(debug=False):
    key = ("dbg" if debug else "main")
    if key not in _cache:
        _cache[key] = build_nc(debug)
    return _cache[key]


def kernel(**inputs):
    h = np.ascontiguousarray(np.asarray(inputs["h"], dtype=np.float32))
    rh = np.ascontiguousarray(np.asarray(inputs["rh"], dtype=np.float32))
    weights = {
        k: np.ascontiguousarray(np.asarray(inputs[k], dtype=np.float32))
        for k in (
            "Wq", "Wk", "Wv", "Wo", "Wrk", "Wrq",
            "W1", "b1", "W2", "b2", "g1", "be1", "g2", "be2",
        )
    }
    in_maps = []
    for c in range(8):
        b, r0 = c // 4, 512 * (c % 4)
        m = {"x": h[b, r0 : r0 + 512, :], "rh": rh[b]}
        m.update(weights)
        in_maps.append(m)

    nc = _get_nc()
    res = run_bass_kernel_spmd(nc, in_maps, core_ids=list(range(8)))
    out = np.empty((B, L, D), dtype=np.float32)
    for c in range(8):
        b, r0 = c // 4, 512 * (c % 4)
        out[b, r0 : r0 + 512, :] = res.results[c]["out"]
    return out

